# revision 17
# baseline (speedup 1.0000x reference)
"""Self-attention (SAGAN-style) Trainium2 kernel.

Reference computation (per batch sample):
    theta = w_theta @ x            # [32, 4096]
    phi   = pool2x2(w_phi @ x)     # [32, 1024]
    g     = pool2x2(w_g @ x)       # [128, 1024]
    beta  = softmax(theta.T @ phi, axis=-1)   # [4096, 1024]
    attn  = g @ beta.T             # [128, 4096]
    out   = gamma * (w_o @ attn) + x

Sharding: data-parallel over batch; B=16 over 8 cores -> 2 samples/core.

Kernel strategy (per core, per sample), all matmuls bf16 (fp32 PSUM accum):
  - x loaded via gpsimd casting DMA straight to bf16; the fp32 x needed for
    the residual is re-DMAed in [128,512] chunks at consume time.
  - one combined projection weight [256, 128] computes theta twice and phi
    twice (rows 0:32/32:64 theta, 64:96/96:128 phi) so the K=32 score matmuls
    can run 2-way row-tiled (tile_position (0,0)/(32,0)).
  - scoresT in [m, n] layout; exp on ScalarE straight out of PSUM -> bf16
    SBUF (logits are O(+-40): exp without max-subtraction is safe). The
    score/exp work for quarter qt is emitted together with the attention
    for quarter qt-1 so the PE has matmul work while ScalarE exps.
  - attn[c, n] = sum_mc gT[mc].T @ expT[mc]; gT from PE transposes of pooled
    g. The softmax denominator rides the same rhs streams through an all-ones
    stationary operand, which also broadcasts the row-sum to all partitions.
  - normalize via reciprocal_approx_fast + scalar_tensor_tensor;
    o = (gamma*w_o).T @ attn; residual fused into PSUM evacuation.
"""

import numpy as np

import concourse.bacc as bacc
import concourse.mybir as mybir
from concourse import tile
from concourse.bass_utils import run_bass_kernel_spmd

F32 = mybir.dt.float32
BF16 = mybir.dt.bfloat16

B, C, H, W = 16, 256, 64, 64
N = H * W            # 4096
M = N // 4           # 1024
C8 = C // 8          # 32
C2 = C // 2          # 128
NCORES = 8
BPC = B // NCORES    # 2 samples per core
NCH = 512            # n-chunk width for matmul streaming
NNCH = N // NCH      # 8
MC = M // 128        # 8 m-chunks


def build_kernel():
    nc = bacc.Bacc("TRN2", target_bir_lowering=False, debug=False)

    x_d = nc.declare_dram_parameter("x", [BPC, C, N], F32, isOutput=False)
    # [cc][128 chans][th th ph ph] and [cc][128 chans][g]
    wq_d = nc.declare_dram_parameter("wq", [2, 128, 128], F32, isOutput=False)
    wg_d = nc.declare_dram_parameter("wg", [2, 128, C2], F32, isOutput=False)
    wo_d = nc.declare_dram_parameter("wo", [C2, C], F32, isOutput=False)  # (gamma*w_o).T
    id_d = nc.declare_dram_parameter("ident", [128, 128], F32, isOutput=False)
    out_d = nc.declare_dram_parameter("out", [BPC, C, N], F32, isOutput=True)

    with tile.TileContext(nc) as tc:
        with (
            tc.tile_pool(name="const", bufs=1) as constp,
            tc.tile_pool(name="xbf", bufs=4) as xbfp,
            tc.tile_pool(name="xres", bufs=6) as xrp,
            tc.tile_pool(name="proj", bufs=2) as projp,
            tc.tile_pool(name="exp", bufs=1) as expp,
            tc.tile_pool(name="gt", bufs=2) as gtp,
            tc.tile_pool(name="small", bufs=3) as smallp,
            tc.tile_pool(name="outs", bufs=4) as outp,
            tc.tile_pool(name="ps_big", bufs=3, space="PSUM") as psb,
            tc.tile_pool(name="ps_a", bufs=1, space="PSUM") as psa,
            tc.tile_pool(name="ps_d", bufs=1, space="PSUM") as psd,
        ):
            # ---- constants / weights (loaded once, cast by DMA) ----
            wq, wg = [], []
            for cc in range(2):
                t = constp.tile([128, 128], BF16, tag=f"wq{cc}")
                nc.gpsimd.dma_start(t[:], wq_d[cc])
                wq.append(t)
                t = constp.tile([128, C2], BF16, tag=f"wg{cc}")
                nc.gpsimd.dma_start(t[:], wg_d[cc])
                wg.append(t)
            wo = constp.tile([C2, C], BF16, tag="wo")
            nc.gpsimd.dma_start(wo[:], wo_d[:])
            id_b = constp.tile([128, 128], BF16, tag="id_b")
            nc.gpsimd.dma_start(id_b[:], id_d[:])
            ones = constp.tile([128, 128], BF16, tag="ones")
            nc.gpsimd.memset(ones[:], 1.0)

            # ---- per-batch state; x DMAs issued up front ----
            st = {}
            for b in range(BPC):
                s = {}
                s["xbs"] = []
                for cc in range(2):
                    xb = xbfp.tile([128, N], BF16, tag="xb", name=f"xb{b}_{cc}")
                    nc.gpsimd.dma_start(xb[:], x_d[b, cc * 128:(cc + 1) * 128, :])
                    s["xbs"].append(xb)
                st[b] = s

            def emit_proj_round(b, i):
                s = st[b]
                if i == 0:
                    s["thph"] = projp.tile([128, N], BF16, tag="thph",
                                           name=f"thph{b}")
                    s["g_sb"] = projp.tile([C2, N], BF16, tag="g_sb",
                                           name=f"g_sb{b}")
                sl = slice(i * NCH, (i + 1) * NCH)
                ps1 = psb.tile([128, NCH], F32, tag="big", name=f"ps1_{b}_{i}")
                for cc in range(2):
                    nc.tensor.matmul(ps1[:], wq[cc][:], s["xbs"][cc][:, sl],
                                     start=(cc == 0), stop=(cc == 1))
                nc.scalar.copy(s["thph"][:, sl], ps1[:])
                ps2 = psb.tile([128, NCH], F32, tag="big", name=f"ps2_{b}_{i}")
                for cc in range(2):
                    nc.tensor.matmul(ps2[:], wg[cc][:], s["xbs"][cc][:, sl],
                                     start=(cc == 0), stop=(cc == 1))
                nc.vector.tensor_copy(s["g_sb"][:, sl], ps2[:])

            def emit_pools(b):
                s = st[b]
                thph, g_sb = s["thph"], s["g_sb"]
                ph2t = projp.tile([64, N // 2], BF16, tag="ph2t", name=f"ph2t{b}")
                pv = thph[:].rearrange("p (w2 two) -> p w2 two", two=2)
                nc.vector.tensor_max(ph2t[:], pv[64:128, :, 0], pv[64:128, :, 1])
                ph2 = projp.tile([64, M], BF16, tag="ph2", name=f"ph2_{b}")
                v2 = ph2t[:].rearrange("p (h2 hb w2) -> p h2 w2 hb",
                                       h2=H // 2, hb=2, w2=W // 2)
                nc.vector.tensor_max(ph2[:], v2[:, :, :, 0], v2[:, :, :, 1])
                g_t = projp.tile([C2, N // 2], BF16, tag="g_t", name=f"g_t{b}")
                pv2 = g_sb[:].rearrange("p (w2 two) -> p w2 two", two=2)
                nc.vector.tensor_max(g_t[:], pv2[:, :, 0], pv2[:, :, 1])
                gp = projp.tile([C2, M], BF16, tag="g_p", name=f"gp{b}")
                v2 = g_t[:].rearrange("p (h2 hb w2) -> p h2 w2 hb",
                                      h2=H // 2, hb=2, w2=W // 2)
                nc.vector.tensor_max(gp[:], v2[:, :, :, 0], v2[:, :, :, 1])
                s["ph2"] = ph2
                s["gp"] = gp
                s["gts"] = []
                s["ets"] = []
                for mc in range(MC):
                    et = expp.tile([128, N], BF16, tag=f"expT{mc}",
                                   name=f"expT{mc}_{b}")
                    s["ets"].append(et)

            def emit_transpose(b, mc):
                s = st[b]
                tp = psa.tile([128, 128], BF16, tag="a", name=f"tp{b}_{mc}")
                nc.tensor.transpose(tp[:], s["gp"][:, mc * 128:(mc + 1) * 128],
                                    id_b[:])
                gt = gtp.tile([128, 128], BF16, tag=f"gt{mc}", name=f"gt{mc}_{b}")
                nc.vector.tensor_copy(gt[:], tp[:])
                s["gts"].append(gt)

            def emit_scores_round(b, qt, r):
                s = st[b]
                th2, ph2, ets = s["thph"][0:64], s["ph2"], s["ets"]
                qsl = slice(qt * 1024, (qt + 1) * 1024)
                mc_a, mc_b = 2 * r, 2 * r + 1
                spa = psb.tile([128, 1024], F32, tag="big", name=f"spa{b}_{qt}_{r}")
                spb = psb.tile([128, 1024], F32, tag="big", name=f"spb{b}_{qt}_{r}")
                for hf in range(2):
                    nsl = slice(qt * 1024 + hf * 512, qt * 1024 + (hf + 1) * 512)
                    osl = slice(hf * 512, (hf + 1) * 512)
                    nc.tensor.matmul(
                        spa[:, osl], ph2[0:32, mc_a * 128:(mc_a + 1) * 128],
                        th2[0:32, nsl], start=True, stop=True)
                    nc.tensor.matmul(
                        spb[:, osl], ph2[32:64, mc_b * 128:(mc_b + 1) * 128],
                        th2[32:64, nsl], start=True, stop=True)
                nc.scalar.activation(ets[mc_a][:, qsl], spa[:],
                                     mybir.ActivationFunctionType.Exp)
                nc.scalar.activation(ets[mc_b][:, qsl], spb[:],
                                     mybir.ActivationFunctionType.Exp)

            aps_map = {}

            def emit_unit_attn(b, i):
                s = st[b]
                nsl = slice(i * NCH, (i + 1) * NCH)
                aps = psa.tile([128, NCH], F32, tag="a", name=f"aps{b}_{i}")
                aps_map[(b, i)] = aps
                for mc in range(MC):
                    nc.tensor.matmul(aps[:], s["gts"][mc][:], s["ets"][mc][:, nsl],
                                     start=(mc == 0), stop=(mc == MC - 1),
                                     skip_group_check=True)

            def emit_unit_den_epi(b, i):
                s = st[b]
                nsl = slice(i * NCH, (i + 1) * NCH)
                aps = aps_map.pop((b, i))
                dps = psd.tile([128, NCH], F32, tag="d", name=f"dps{b}_{i}")
                xr = xrp.tile([128, 1024], F32, tag="xr", name=f"xr{b}_{i}")
                for oc in range(2):
                    nc.sync.dma_start(xr[:, oc * NCH:(oc + 1) * NCH],
                                      x_d[b, oc * 128:(oc + 1) * 128, nsl])
                for mc in range(MC):
                    nc.tensor.matmul(dps[:], ones[:], s["ets"][mc][:, nsl],
                                     start=(mc == 0), stop=(mc == MC - 1),
                                     skip_group_check=True)
                rec = smallp.tile([128, NCH], F32, tag="rec", name=f"rec{b}_{i}")
                nc.vector.reciprocal_approx_fast(rec[:], dps[:])
                at = smallp.tile([128, NCH], BF16, tag="attn", name=f"at{b}_{i}")
                nc.vector.scalar_tensor_tensor(
                    at[:], aps[:], 1.0, rec[:],
                    mybir.AluOpType.bypass, mybir.AluOpType.mult)
                op0 = psa.tile([128, NCH], F32, tag="a", name=f"op0_{b}_{i}")
                nc.tensor.matmul(op0[:], wo[:, 0:128], at[:], start=True, stop=True)
                op1 = psd.tile([128, NCH], F32, tag="d", name=f"op1_{b}_{i}")
                nc.tensor.matmul(op1[:], wo[:, 128:256], at[:], start=True, stop=True)
                for oc, ops in ((0, op0), (1, op1)):
                    osb = outp.tile([128, NCH], F32, tag="osb",
                                    name=f"osb{b}_{i}_{oc}")
                    nc.vector.scalar_tensor_tensor(
                        osb[:], ops[:], 1.0, xr[:, oc * NCH:(oc + 1) * NCH],
                        mybir.AluOpType.bypass, mybir.AluOpType.add)
                    nc.sync.dma_start(out_d[b, oc * 128:(oc + 1) * 128, nsl],
                                      osb[:])

            # ---- global pipelined emission ----
            # filler queue: (ready_round, weight, closure); rounds are numbered
            # 0..15 for b0's score rounds, 16..31 for b1's.
            fillers = []
            for qt in range(4):
                for i in (2 * qt, 2 * qt + 1):
                    fillers.append((4 * (qt + 1), 8, lambda b=0, i=i: emit_unit_attn(b, i)))
                    fillers.append((4 * (qt + 1), 8, lambda b=0, i=i: emit_unit_den_epi(b, i)))
            for i in range(NNCH):
                fillers.append((6, 4, lambda i=i: emit_proj_round(1, i)))
            fillers.append((6, 0, lambda: emit_pools(1)))
            for mc in range(MC):
                fillers.append((8, 1, lambda mc=mc: emit_transpose(1, mc)))
            for qt in range(4):
                for i in (2 * qt, 2 * qt + 1):
                    fillers.append((16 + 4 * (qt + 1), 8, lambda b=1, i=i: emit_unit_attn(b, i)))
                    fillers.append((16 + 4 * (qt + 1), 8, lambda b=1, i=i: emit_unit_den_epi(b, i)))

            fidx = 0

            def pop_fillers(rnd, budget):
                nonlocal fidx
                while fidx < len(fillers):
                    ready, weight, fn = fillers[fidx]
                    if ready > rnd or weight > budget:
                        break
                    fn()
                    fidx += 1
                    budget -= weight

            for i in range(NNCH):
                emit_proj_round(0, i)
            emit_pools(0)
            for mc in range(MC):
                emit_transpose(0, mc)

            rnd = 0
            for b in range(BPC):
                for qt in range(4):
                    for r in range(4):
                        emit_scores_round(b, qt, r)
                        rnd += 1
                        pop_fillers(rnd, 10)
                if b == 0:
                    pop_fillers(16, 10 ** 6)  # drain b0 units + b1 pre-work
            pop_fillers(10 ** 6, 10 ** 6)     # tail: b1's last-quarter units

    nc.compile()
    return nc


_NC_CACHE = None


def _get_nc():
    global _NC_CACHE
    if _NC_CACHE is None:
        _NC_CACHE = build_kernel()
    return _NC_CACHE


def prep_inputs(x, w_theta, w_phi, w_g, w_o, gamma):
    """Host-side prep: shard x over 8 cores; transpose/scale/pack weights."""
    x = np.asarray(x, dtype=np.float32).reshape(B, C, N)
    w_theta = np.asarray(w_theta, dtype=np.float32)
    w_phi = np.asarray(w_phi, dtype=np.float32)
    w_g = np.asarray(w_g, dtype=np.float32)
    w_o = np.asarray(w_o, dtype=np.float32)
    gamma = np.float32(gamma)

    # combined projection weight: [th th ph ph] along output dim
    wqT = np.concatenate([w_theta.T, w_theta.T, w_phi.T, w_phi.T], axis=1)  # [256,128]
    wq = np.ascontiguousarray(wqT.reshape(2, 128, 128))
    wgq = np.ascontiguousarray(w_g.T.reshape(2, 128, C2))
    wo = np.ascontiguousarray((gamma * w_o).T)
    ident = np.eye(128, dtype=np.float32)

    in_maps = []
    for core in range(NCORES):
        shard = np.ascontiguousarray(x[core * BPC:(core + 1) * BPC])
        in_maps.append({"x": shard, "wq": wq, "wg": wgq, "wo": wo, "ident": ident})
    return in_maps


def run(inputs, trace=False, **kw):
    nc = _get_nc()
    in_maps = prep_inputs(**inputs)
    res = run_bass_kernel_spmd(nc, in_maps, core_ids=list(range(NCORES)),
                               trace=trace, **kw)
    outs = [res.results[i]["out"] for i in range(NCORES)]
    full = np.concatenate(outs, axis=0).reshape(B, C, H, W).astype(np.float32)
    return full, res


def kernel(**inputs):
    full, _ = run(inputs, trace=False)
    return full


# revision 18
# speedup vs baseline: 1.0875x; 1.0875x over previous
"""Self-attention (SAGAN-style) Trainium2 kernel.

Reference computation (per batch sample):
    theta = w_theta @ x            # [32, 4096]
    phi   = pool2x2(w_phi @ x)     # [32, 1024]
    g     = pool2x2(w_g @ x)       # [128, 1024]
    beta  = softmax(theta.T @ phi, axis=-1)   # [4096, 1024]
    attn  = g @ beta.T             # [128, 4096]
    out   = gamma * (w_o @ attn) + x

Sharding: data-parallel over batch; B=16 over 8 cores -> 2 samples/core.

Kernel strategy (per core, per sample), all matmuls bf16 (fp32 PSUM accum):
  - x loaded via gpsimd casting DMA straight to bf16; the fp32 x needed for
    the residual is re-DMAed in [128,512] chunks at consume time.
  - one combined projection weight [256, 128] computes theta twice and phi
    twice (rows 0:32/32:64 theta, 64:96/96:128 phi) so the K=32 score matmuls
    can run 2-way row-tiled (tile_position (0,0)/(32,0)).
  - scoresT in [m, n] layout; exp on ScalarE straight out of PSUM -> bf16
    SBUF (logits are O(+-40): exp without max-subtraction is safe). The
    score/exp work for quarter qt is emitted together with the attention
    for quarter qt-1 so the PE has matmul work while ScalarE exps.
  - attn[c, n] = sum_mc gT[mc].T @ expT[mc]; gT from PE transposes of pooled
    g. The softmax denominator rides the same rhs streams through an all-ones
    stationary operand, which also broadcasts the row-sum to all partitions.
  - normalize via reciprocal_approx_fast + scalar_tensor_tensor;
    o = (gamma*w_o).T @ attn; residual fused into PSUM evacuation.
"""

import numpy as np

import concourse.bacc as bacc
import concourse.mybir as mybir
from concourse import tile
from concourse.bass_utils import run_bass_kernel_spmd

F32 = mybir.dt.float32
BF16 = mybir.dt.bfloat16

B, C, H, W = 16, 256, 64, 64
N = H * W            # 4096
M = N // 4           # 1024
C8 = C // 8          # 32
C2 = C // 2          # 128
NCORES = 8
BPC = B // NCORES    # 2 samples per core
NCH = 512            # n-chunk width for matmul streaming
NNCH = N // NCH      # 8
MC = M // 128        # 8 m-chunks


def build_kernel():
    nc = bacc.Bacc("TRN2", target_bir_lowering=False, debug=False)

    x_d = nc.declare_dram_parameter("x", [BPC, C, N], F32, isOutput=False)
    # [cc][128 chans][th th ph ph] and [cc][128 chans][g]
    wq_d = nc.declare_dram_parameter("wq", [2, 128, 128], F32, isOutput=False)
    wg_d = nc.declare_dram_parameter("wg", [2, 128, C2], F32, isOutput=False)
    wo_d = nc.declare_dram_parameter("wo", [C2, C], F32, isOutput=False)  # (gamma*w_o).T
    id_d = nc.declare_dram_parameter("ident", [128, 128], F32, isOutput=False)
    out_d = nc.declare_dram_parameter("out", [BPC, C, N], F32, isOutput=True)

    with tile.TileContext(nc) as tc:
        with (
            tc.tile_pool(name="const", bufs=1) as constp,
            tc.tile_pool(name="xbf", bufs=4) as xbfp,
            tc.tile_pool(name="xres", bufs=6) as xrp,
            tc.tile_pool(name="proj", bufs=2) as projp,
            tc.tile_pool(name="exp", bufs=1) as expp,
            tc.tile_pool(name="gt", bufs=1) as gtp,
            tc.tile_pool(name="small", bufs=3) as smallp,
            tc.tile_pool(name="outs", bufs=4) as outp,
            tc.tile_pool(name="ps_big", bufs=3, space="PSUM") as psb,
            tc.tile_pool(name="ps_a", bufs=1, space="PSUM") as psa,
            tc.tile_pool(name="ps_d", bufs=1, space="PSUM") as psd,
        ):
            # ---- constants / weights (loaded once, cast by DMA) ----
            wq, wg = [], []
            for cc in range(2):
                t = constp.tile([128, 128], BF16, tag=f"wq{cc}")
                nc.gpsimd.dma_start(t[:], wq_d[cc])
                wq.append(t)
                t = constp.tile([128, C2], BF16, tag=f"wg{cc}")
                nc.gpsimd.dma_start(t[:], wg_d[cc])
                wg.append(t)
            wo = constp.tile([C2, C], BF16, tag="wo")
            nc.gpsimd.dma_start(wo[:], wo_d[:])
            id_b = constp.tile([128, 128], BF16, tag="id_b")
            nc.gpsimd.dma_start(id_b[:], id_d[:])
            ones = constp.tile([128, 128], BF16, tag="ones")
            nc.gpsimd.memset(ones[:], 1.0)

            for b in range(BPC):
                # ---- load x as bf16 (casting DMA on gpsimd SWDGE) ----
                xbs = []
                for cc in range(2):
                    xb = xbfp.tile([128, N], BF16, tag="xb", name=f"xb{b}_{cc}")
                    nc.gpsimd.dma_start(xb[:], x_d[b, cc * 128:(cc + 1) * 128, :])
                    xbs.append(xb)

                # ---- projections ----
                thph = projp.tile([128, N], BF16, tag="thph")  # 0:64 dup-theta, 64:128 dup-phi
                g_sb = projp.tile([C2, N], BF16, tag="g_sb")
                for i in range(NNCH):
                    sl = slice(i * NCH, (i + 1) * NCH)
                    ps1 = psa.tile([128, NCH], F32, tag="a", name=f"ps1_{b}_{i}")
                    for cc in range(2):
                        nc.tensor.matmul(ps1[:], wq[cc][:], xbs[cc][:, sl],
                                         start=(cc == 0), stop=(cc == 1))
                    nc.scalar.copy(thph[:, sl], ps1[:])
                    ps2 = psd.tile([128, NCH], F32, tag="d", name=f"ps2_{b}_{i}")
                    for cc in range(2):
                        nc.tensor.matmul(ps2[:], wg[cc][:], xbs[cc][:, sl],
                                         start=(cc == 0), stop=(cc == 1))
                    nc.scalar.copy(g_sb[:, sl], ps2[:])
                th2 = thph[0:64]

                # ---- 2x2 maxpool (w-pairs then h-pairs, strided SBUF ops) ----
                ph2t = projp.tile([64, N // 2], BF16, tag="ph2t")
                pv = thph[:].rearrange("p (w2 two) -> p w2 two", two=2)
                nc.vector.tensor_max(ph2t[:], pv[64:128, :, 0], pv[64:128, :, 1])
                ph2 = projp.tile([64, M], BF16, tag="ph2")
                v2 = ph2t[:].rearrange("p (h2 hb w2) -> p h2 w2 hb", h2=H // 2, hb=2, w2=W // 2)
                nc.vector.tensor_max(ph2[:], v2[:, :, :, 0], v2[:, :, :, 1])
                g_t = projp.tile([C2, N // 2], BF16, tag="g_t")
                pv2 = g_sb[:].rearrange("p (w2 two) -> p w2 two", two=2)
                nc.vector.tensor_max(g_t[:], pv2[:, :, 0], pv2[:, :, 1])
                gp = projp.tile([C2, M], BF16, tag="g_p")
                v2 = g_t[:].rearrange("p (h2 hb w2) -> p h2 w2 hb", h2=H // 2, hb=2, w2=W // 2)
                nc.vector.tensor_max(gp[:], v2[:, :, :, 0], v2[:, :, :, 1])

                # ---- gT: transpose pooled g into 8 [128m, 128c] chunks ----
                gts = []
                for mc in range(MC):
                    tp = psa.tile([128, 128], BF16, tag="a", name=f"tp{b}_{mc}")
                    nc.tensor.transpose(tp[:], gp[:, mc * 128:(mc + 1) * 128], id_b[:])
                    gt = gtp.tile([128, 128], BF16, tag=f"gt{mc}", name=f"gt{mc}_{b}")
                    nc.vector.tensor_copy(gt[:], tp[:])
                    gts.append(gt)

                # ---- scores/exp interleaved with attention at round granularity ----
                # PE executes its stream in order, so alternate 4 score MMs
                # (one round) with 8 attention-accumulation MMs; ScalarE exps
                # overlap the attention matmuls.
                ets = []
                for mc in range(MC):
                    et = expp.tile([128, N], BF16, tag=f"expT{mc}", name=f"expT{mc}_{b}")
                    ets.append(et)

                # deferred attention work-units, 8 accumulation MMs each:
                # (chunk, 'attn') and (chunk, 'den') + epilogue after 'den'
                aps_map = {}

                def unit_attn(i):
                    nsl = slice(i * NCH, (i + 1) * NCH)
                    aps = psa.tile([128, NCH], F32, tag="a", name=f"aps{b}_{i}")
                    aps_map[i] = aps
                    for mc in range(MC):
                        nc.tensor.matmul(aps[:], gts[mc][:], ets[mc][:, nsl],
                                         start=(mc == 0), stop=(mc == MC - 1),
                                         skip_group_check=True)

                def unit_den_epi(i):
                    nsl = slice(i * NCH, (i + 1) * NCH)
                    aps = aps_map.pop(i)
                    dps = psd.tile([128, NCH], F32, tag="d", name=f"dps{b}_{i}")
                    xr = xrp.tile([128, 1024], F32, tag="xr", name=f"xr{b}_{i}")
                    for oc in range(2):
                        nc.sync.dma_start(xr[:, oc * NCH:(oc + 1) * NCH],
                                          x_d[b, oc * 128:(oc + 1) * 128, nsl])
                    for mc in range(MC):
                        nc.tensor.matmul(dps[:], ones[:], ets[mc][:, nsl],
                                         start=(mc == 0), stop=(mc == MC - 1),
                                         skip_group_check=True)
                    rec = smallp.tile([128, NCH], F32, tag="rec", name=f"rec{b}_{i}")
                    nc.vector.reciprocal_approx_fast(rec[:], dps[:])
                    at = smallp.tile([128, NCH], BF16, tag="attn", name=f"at{b}_{i}")
                    nc.vector.scalar_tensor_tensor(
                        at[:], aps[:], 1.0, rec[:],
                        mybir.AluOpType.bypass, mybir.AluOpType.mult)
                    op0 = psa.tile([128, NCH], F32, tag="a", name=f"op0_{b}_{i}")
                    nc.tensor.matmul(op0[:], wo[:, 0:128], at[:], start=True, stop=True)
                    op1 = psd.tile([128, NCH], F32, tag="d", name=f"op1_{b}_{i}")
                    nc.tensor.matmul(op1[:], wo[:, 128:256], at[:], start=True, stop=True)
                    for oc, ops in ((0, op0), (1, op1)):
                        osb = outp.tile([128, NCH], F32, tag="osb",
                                        name=f"osb{b}_{i}_{oc}")
                        nc.vector.scalar_tensor_tensor(
                            osb[:], ops[:], 1.0, xr[:, oc * NCH:(oc + 1) * NCH],
                            mybir.AluOpType.bypass, mybir.AluOpType.add)
                        nc.sync.dma_start(out_d[b, oc * 128:(oc + 1) * 128, nsl],
                                          osb[:])

                units = []
                for i in range(NNCH):
                    units.append(lambda i=i: unit_attn(i))
                    units.append(lambda i=i: unit_den_epi(i))
                uidx = 0

                for qt in range(5):
                    if qt < 4:
                        qsl = slice(qt * 1024, (qt + 1) * 1024)
                        for r in range(4):
                            mc_a, mc_b = 2 * r, 2 * r + 1
                            spa = psb.tile([128, 1024], F32, tag="big",
                                           name=f"spa{b}_{qt}_{r}")
                            spb = psb.tile([128, 1024], F32, tag="big",
                                           name=f"spb{b}_{qt}_{r}")
                            for hf in range(2):
                                nsl = slice(qt * 1024 + hf * 512, qt * 1024 + (hf + 1) * 512)
                                osl = slice(hf * 512, (hf + 1) * 512)
                                nc.tensor.matmul(
                                    spa[:, osl], ph2[0:32, mc_a * 128:(mc_a + 1) * 128],
                                    th2[0:32, nsl], start=True, stop=True)
                                nc.tensor.matmul(
                                    spb[:, osl], ph2[32:64, mc_b * 128:(mc_b + 1) * 128],
                                    th2[32:64, nsl], start=True, stop=True)
                            nc.scalar.activation(ets[mc_a][:, qsl], spa[:],
                                                 mybir.ActivationFunctionType.Exp)
                            nc.scalar.activation(ets[mc_b][:, qsl], spb[:],
                                                 mybir.ActivationFunctionType.Exp)
                            if qt >= 1 and uidx < len(units):
                                units[uidx](); uidx += 1
                    else:
                        while uidx < len(units):
                            units[uidx](); uidx += 1

    nc.compile()
    return nc


_NC_CACHE = None


def _get_nc():
    global _NC_CACHE
    if _NC_CACHE is None:
        _NC_CACHE = build_kernel()
    return _NC_CACHE


def prep_inputs(x, w_theta, w_phi, w_g, w_o, gamma):
    """Host-side prep: shard x over 8 cores; transpose/scale/pack weights."""
    x = np.asarray(x, dtype=np.float32).reshape(B, C, N)
    w_theta = np.asarray(w_theta, dtype=np.float32)
    w_phi = np.asarray(w_phi, dtype=np.float32)
    w_g = np.asarray(w_g, dtype=np.float32)
    w_o = np.asarray(w_o, dtype=np.float32)
    gamma = np.float32(gamma)

    # combined projection weight: [th th ph ph] along output dim
    wqT = np.concatenate([w_theta.T, w_theta.T, w_phi.T, w_phi.T], axis=1)  # [256,128]
    wq = np.ascontiguousarray(wqT.reshape(2, 128, 128))
    wgq = np.ascontiguousarray(w_g.T.reshape(2, 128, C2))
    wo = np.ascontiguousarray((gamma * w_o).T)
    ident = np.eye(128, dtype=np.float32)

    in_maps = []
    for core in range(NCORES):
        shard = np.ascontiguousarray(x[core * BPC:(core + 1) * BPC])
        in_maps.append({"x": shard, "wq": wq, "wg": wgq, "wo": wo, "ident": ident})
    return in_maps


def run(inputs, trace=False, **kw):
    nc = _get_nc()
    in_maps = prep_inputs(**inputs)
    res = run_bass_kernel_spmd(nc, in_maps, core_ids=list(range(NCORES)),
                               trace=trace, **kw)
    outs = [res.results[i]["out"] for i in range(NCORES)]
    full = np.concatenate(outs, axis=0).reshape(B, C, H, W).astype(np.float32)
    return full, res


def kernel(**inputs):
    full, _ = run(inputs, trace=False)
    return full


# revision 19
# speedup vs baseline: 1.1897x; 1.0940x over previous
"""Self-attention (SAGAN-style) Trainium2 kernel.

Reference computation (per batch sample):
    theta = w_theta @ x            # [32, 4096]
    phi   = pool2x2(w_phi @ x)     # [32, 1024]
    g     = pool2x2(w_g @ x)       # [128, 1024]
    beta  = softmax(theta.T @ phi, axis=-1)   # [4096, 1024]
    attn  = g @ beta.T             # [128, 4096]
    out   = gamma * (w_o @ attn) + x

Sharding: data-parallel over batch; B=16 over 8 cores -> 2 samples/core.

Kernel strategy (per core, per sample), all matmuls bf16 (fp32 PSUM accum):
  - x loaded via gpsimd casting DMA straight to bf16; the fp32 x needed for
    the residual is re-DMAed in [128,512] chunks at consume time.
  - one combined projection weight [256, 128] computes theta twice and phi
    twice (rows 0:32/32:64 theta, 64:96/96:128 phi) so the K=32 score matmuls
    can run 2-way row-tiled (tile_position (0,0)/(32,0)).
  - scoresT in [m, n] layout; exp on ScalarE straight out of PSUM -> bf16
    SBUF (logits are O(+-40): exp without max-subtraction is safe). The
    score/exp work for quarter qt is emitted together with the attention
    for quarter qt-1 so the PE has matmul work while ScalarE exps.
  - attn[c, n] = sum_mc gT[mc].T @ expT[mc]; gT from PE transposes of pooled
    g. The softmax denominator rides the same rhs streams through an all-ones
    stationary operand, which also broadcasts the row-sum to all partitions.
  - normalize via reciprocal_approx_fast + scalar_tensor_tensor;
    o = (gamma*w_o).T @ attn; residual fused into PSUM evacuation.
"""

import numpy as np

import concourse.bacc as bacc
import concourse.mybir as mybir
from concourse import tile
from concourse.bass_utils import run_bass_kernel_spmd

F32 = mybir.dt.float32
BF16 = mybir.dt.bfloat16

B, C, H, W = 16, 256, 64, 64
N = H * W            # 4096
M = N // 4           # 1024
C8 = C // 8          # 32
C2 = C // 2          # 128
NCORES = 8
BPC = B // NCORES    # 2 samples per core
NCH = 512            # n-chunk width for matmul streaming
NNCH = N // NCH      # 8
MC = M // 128        # 8 m-chunks


def build_kernel():
    nc = bacc.Bacc("TRN2", target_bir_lowering=False, debug=False)

    x_d = nc.declare_dram_parameter("x", [BPC, C, N], F32, isOutput=False)
    # [cc][128 chans][th th ph ph] and [cc][128 chans][g]
    wq_d = nc.declare_dram_parameter("wq", [2, 128, 128], F32, isOutput=False)
    wg_d = nc.declare_dram_parameter("wg", [2, 128, C2], F32, isOutput=False)
    wo_d = nc.declare_dram_parameter("wo", [C2, C], F32, isOutput=False)  # (gamma*w_o).T
    id_d = nc.declare_dram_parameter("ident", [128, 128], F32, isOutput=False)
    out_d = nc.declare_dram_parameter("out", [BPC, C, N], F32, isOutput=True)

    with tile.TileContext(nc) as tc:
        with (
            tc.tile_pool(name="const", bufs=1) as constp,
            tc.tile_pool(name="xbf", bufs=4) as xbfp,
            tc.tile_pool(name="xres", bufs=6) as xrp,
            tc.tile_pool(name="proj", bufs=2) as projp,
            tc.tile_pool(name="exp", bufs=1) as expp,
            tc.tile_pool(name="gt", bufs=1) as gtp,
            tc.tile_pool(name="small", bufs=3) as smallp,
            tc.tile_pool(name="outs", bufs=4) as outp,
            tc.tile_pool(name="ps_big", bufs=3, space="PSUM") as psb,
            tc.tile_pool(name="ps_a", bufs=1, space="PSUM") as psa,
            tc.tile_pool(name="ps_d", bufs=1, space="PSUM") as psd,
        ):
            # ---- constants / weights (loaded once, cast by DMA) ----
            wq, wg = [], []
            for cc in range(2):
                t = constp.tile([128, 128], BF16, tag=f"wq{cc}")
                nc.gpsimd.dma_start(t[:], wq_d[cc])
                wq.append(t)
                t = constp.tile([128, C2], BF16, tag=f"wg{cc}")
                nc.gpsimd.dma_start(t[:], wg_d[cc])
                wg.append(t)
            wo = constp.tile([C2, C], BF16, tag="wo")
            nc.gpsimd.dma_start(wo[:], wo_d[:])
            id_b = constp.tile([128, 128], BF16, tag="id_b")
            nc.gpsimd.dma_start(id_b[:], id_d[:])
            ones = constp.tile([128, 128], BF16, tag="ones")
            nc.gpsimd.memset(ones[:], 1.0)

            for b in range(BPC):
                # ---- load x as bf16 (casting DMA on gpsimd SWDGE) ----
                xbs = []
                for cc in range(2):
                    xb = xbfp.tile([128, N], BF16, tag="xb", name=f"xb{b}_{cc}")
                    nc.gpsimd.dma_start(xb[:], x_d[b, cc * 128:(cc + 1) * 128, :])
                    xbs.append(xb)

                # ---- projections ----
                thph = projp.tile([128, N], BF16, tag="thph")  # 0:64 dup-theta, 64:128 dup-phi
                g_sb = projp.tile([C2, N], BF16, tag="g_sb")
                for i in range(NNCH):
                    sl = slice(i * NCH, (i + 1) * NCH)
                    ps1 = psb.tile([128, NCH], F32, tag="big", name=f"ps1_{b}_{i}")
                    for cc in range(2):
                        nc.tensor.matmul(ps1[:], wq[cc][:], xbs[cc][:, sl],
                                         start=(cc == 0), stop=(cc == 1))
                    nc.scalar.copy(thph[:, sl], ps1[:])
                    ps2 = psb.tile([128, NCH], F32, tag="big", name=f"ps2_{b}_{i}")
                    for cc in range(2):
                        nc.tensor.matmul(ps2[:], wg[cc][:], xbs[cc][:, sl],
                                         start=(cc == 0), stop=(cc == 1))
                    nc.scalar.copy(g_sb[:, sl], ps2[:])
                th2 = thph[0:64]

                # ---- 2x2 maxpool (w-pairs then h-pairs, strided SBUF ops) ----
                ph2t = projp.tile([64, N // 2], BF16, tag="ph2t")
                pv = thph[:].rearrange("p (w2 two) -> p w2 two", two=2)
                nc.vector.tensor_max(ph2t[:], pv[64:128, :, 0], pv[64:128, :, 1])
                ph2 = projp.tile([64, M], BF16, tag="ph2")
                v2 = ph2t[:].rearrange("p (h2 hb w2) -> p h2 w2 hb", h2=H // 2, hb=2, w2=W // 2)
                nc.vector.tensor_max(ph2[:], v2[:, :, :, 0], v2[:, :, :, 1])
                g_t = projp.tile([C2, N // 2], BF16, tag="g_t")
                pv2 = g_sb[:].rearrange("p (w2 two) -> p w2 two", two=2)
                nc.vector.tensor_max(g_t[:], pv2[:, :, 0], pv2[:, :, 1])
                gp = projp.tile([C2, M], BF16, tag="g_p")
                v2 = g_t[:].rearrange("p (h2 hb w2) -> p h2 w2 hb", h2=H // 2, hb=2, w2=W // 2)
                nc.vector.tensor_max(gp[:], v2[:, :, :, 0], v2[:, :, :, 1])

                # ---- gT: transpose pooled g into 8 [128m, 128c] chunks ----
                gts = []
                for mc in range(MC):
                    tp = psa.tile([128, 128], BF16, tag="a", name=f"tp{b}_{mc}")
                    nc.tensor.transpose(tp[:], gp[:, mc * 128:(mc + 1) * 128], id_b[:])
                    gt = gtp.tile([128, 128], BF16, tag=f"gt{mc}", name=f"gt{mc}_{b}")
                    nc.vector.tensor_copy(gt[:], tp[:])
                    gts.append(gt)

                # ---- scores/exp interleaved with attention at round granularity ----
                # PE executes its stream in order, so alternate 4 score MMs
                # (one round) with 8 attention-accumulation MMs; ScalarE exps
                # overlap the attention matmuls.
                ets = []
                for mc in range(MC):
                    et = expp.tile([128, N], BF16, tag=f"expT{mc}", name=f"expT{mc}_{b}")
                    ets.append(et)

                # deferred attention work-units, 8 accumulation MMs each:
                # (chunk, 'attn') and (chunk, 'den') + epilogue after 'den'
                aps_map = {}

                def unit_attn(i):
                    nsl = slice(i * NCH, (i + 1) * NCH)
                    aps = psa.tile([128, NCH], F32, tag="a", name=f"aps{b}_{i}")
                    aps_map[i] = aps
                    for mc in range(MC):
                        nc.tensor.matmul(aps[:], gts[mc][:], ets[mc][:, nsl],
                                         start=(mc == 0), stop=(mc == MC - 1),
                                         skip_group_check=True)

                def unit_den_epi(i):
                    nsl = slice(i * NCH, (i + 1) * NCH)
                    aps = aps_map.pop(i)
                    dps = psd.tile([128, NCH], F32, tag="d", name=f"dps{b}_{i}")
                    xr = xrp.tile([128, 1024], F32, tag="xr", name=f"xr{b}_{i}")
                    for oc in range(2):
                        nc.sync.dma_start(xr[:, oc * NCH:(oc + 1) * NCH],
                                          x_d[b, oc * 128:(oc + 1) * 128, nsl])
                    for mc in range(MC):
                        nc.tensor.matmul(dps[:], ones[:], ets[mc][:, nsl],
                                         start=(mc == 0), stop=(mc == MC - 1),
                                         skip_group_check=True)
                    rec = smallp.tile([128, NCH], F32, tag="rec", name=f"rec{b}_{i}")
                    nc.vector.reciprocal_approx_fast(rec[:], dps[:])
                    at = smallp.tile([128, NCH], BF16, tag="attn", name=f"at{b}_{i}")
                    nc.vector.scalar_tensor_tensor(
                        at[:], aps[:], 1.0, rec[:],
                        mybir.AluOpType.bypass, mybir.AluOpType.mult)
                    op0 = psa.tile([128, NCH], F32, tag="a", name=f"op0_{b}_{i}")
                    nc.tensor.matmul(op0[:], wo[:, 0:128], at[:], start=True, stop=True)
                    op1 = psd.tile([128, NCH], F32, tag="d", name=f"op1_{b}_{i}")
                    nc.tensor.matmul(op1[:], wo[:, 128:256], at[:], start=True, stop=True)
                    for oc, ops in ((0, op0), (1, op1)):
                        osb = outp.tile([128, NCH], F32, tag="osb",
                                        name=f"osb{b}_{i}_{oc}")
                        nc.vector.scalar_tensor_tensor(
                            osb[:], ops[:], 1.0, xr[:, oc * NCH:(oc + 1) * NCH],
                            mybir.AluOpType.bypass, mybir.AluOpType.add)
                        nc.sync.dma_start(out_d[b, oc * 128:(oc + 1) * 128, nsl],
                                          osb[:])

                units = []
                for i in range(NNCH):
                    units.append(lambda i=i: unit_attn(i))
                    units.append(lambda i=i: unit_den_epi(i))
                uidx = 0

                for qt in range(5):
                    if qt < 4:
                        qsl = slice(qt * 1024, (qt + 1) * 1024)
                        for r in range(4):
                            mc_a, mc_b = 2 * r, 2 * r + 1
                            spa = psb.tile([128, 1024], F32, tag="big",
                                           name=f"spa{b}_{qt}_{r}")
                            spb = psb.tile([128, 1024], F32, tag="big",
                                           name=f"spb{b}_{qt}_{r}")
                            for hf in range(2):
                                nsl = slice(qt * 1024 + hf * 512, qt * 1024 + (hf + 1) * 512)
                                osl = slice(hf * 512, (hf + 1) * 512)
                                nc.tensor.matmul(
                                    spa[:, osl], ph2[0:32, mc_a * 128:(mc_a + 1) * 128],
                                    th2[0:32, nsl], start=True, stop=True)
                                nc.tensor.matmul(
                                    spb[:, osl], ph2[32:64, mc_b * 128:(mc_b + 1) * 128],
                                    th2[32:64, nsl], start=True, stop=True)
                            nc.scalar.activation(ets[mc_a][:, qsl], spa[:],
                                                 mybir.ActivationFunctionType.Exp)
                            nc.scalar.activation(ets[mc_b][:, qsl], spb[:],
                                                 mybir.ActivationFunctionType.Exp)
                            if qt >= 1 and uidx < len(units):
                                units[uidx](); uidx += 1
                    else:
                        while uidx < len(units):
                            units[uidx](); uidx += 1

    nc.compile()
    return nc


_NC_CACHE = None


def _get_nc():
    global _NC_CACHE
    if _NC_CACHE is None:
        _NC_CACHE = build_kernel()
    return _NC_CACHE


def prep_inputs(x, w_theta, w_phi, w_g, w_o, gamma):
    """Host-side prep: shard x over 8 cores; transpose/scale/pack weights."""
    x = np.asarray(x, dtype=np.float32).reshape(B, C, N)
    w_theta = np.asarray(w_theta, dtype=np.float32)
    w_phi = np.asarray(w_phi, dtype=np.float32)
    w_g = np.asarray(w_g, dtype=np.float32)
    w_o = np.asarray(w_o, dtype=np.float32)
    gamma = np.float32(gamma)

    # combined projection weight: [th th ph ph] along output dim
    wqT = np.concatenate([w_theta.T, w_theta.T, w_phi.T, w_phi.T], axis=1)  # [256,128]
    wq = np.ascontiguousarray(wqT.reshape(2, 128, 128))
    wgq = np.ascontiguousarray(w_g.T.reshape(2, 128, C2))
    wo = np.ascontiguousarray((gamma * w_o).T)
    ident = np.eye(128, dtype=np.float32)

    in_maps = []
    for core in range(NCORES):
        shard = np.ascontiguousarray(x[core * BPC:(core + 1) * BPC])
        in_maps.append({"x": shard, "wq": wq, "wg": wgq, "wo": wo, "ident": ident})
    return in_maps


def run(inputs, trace=False, **kw):
    nc = _get_nc()
    in_maps = prep_inputs(**inputs)
    res = run_bass_kernel_spmd(nc, in_maps, core_ids=list(range(NCORES)),
                               trace=trace, **kw)
    outs = [res.results[i]["out"] for i in range(NCORES)]
    full = np.concatenate(outs, axis=0).reshape(B, C, H, W).astype(np.float32)
    return full, res


def kernel(**inputs):
    full, _ = run(inputs, trace=False)
    return full


# revision 24
# speedup vs baseline: 1.1968x; 1.0060x over previous
"""Self-attention (SAGAN-style) Trainium2 kernel.

Reference computation (per batch sample):
    theta = w_theta @ x            # [32, 4096]
    phi   = pool2x2(w_phi @ x)     # [32, 1024]
    g     = pool2x2(w_g @ x)       # [128, 1024]
    beta  = softmax(theta.T @ phi, axis=-1)   # [4096, 1024]
    attn  = g @ beta.T             # [128, 4096]
    out   = gamma * (w_o @ attn) + x

Sharding: data-parallel over batch; B=16 over 8 cores -> 2 samples/core.

Kernel strategy (per core, per sample), all matmuls bf16 (fp32 PSUM accum):
  - x loaded via gpsimd casting DMA straight to bf16; the fp32 x needed for
    the residual is re-DMAed in [128,512] chunks at consume time.
  - one combined projection weight [256, 128] computes theta twice and phi
    twice (rows 0:32/32:64 theta, 64:96/96:128 phi) so the K=32 score matmuls
    can run 2-way row-tiled (tile_position (0,0)/(32,0)).
  - scoresT in [m, n] layout; exp on ScalarE straight out of PSUM -> bf16
    SBUF (logits are O(+-40): exp without max-subtraction is safe). The
    score/exp work for quarter qt is emitted together with the attention
    for quarter qt-1 so the PE has matmul work while ScalarE exps.
  - attn[c, n] = sum_mc gT[mc].T @ expT[mc]; gT from PE transposes of pooled
    g. The softmax denominator rides the same rhs streams through an all-ones
    stationary operand, which also broadcasts the row-sum to all partitions.
  - normalize via reciprocal_approx_fast + scalar_tensor_tensor;
    o = (gamma*w_o).T @ attn; residual fused into PSUM evacuation.
"""

import numpy as np

import concourse.bacc as bacc
import concourse.mybir as mybir
from concourse import tile
from concourse.bass_utils import run_bass_kernel_spmd

F32 = mybir.dt.float32
BF16 = mybir.dt.bfloat16

B, C, H, W = 16, 256, 64, 64
N = H * W            # 4096
M = N // 4           # 1024
C8 = C // 8          # 32
C2 = C // 2          # 128
NCORES = 8
BPC = B // NCORES    # 2 samples per core
NCH = 512            # n-chunk width for matmul streaming
NNCH = N // NCH      # 8
MC = M // 128        # 8 m-chunks


def build_kernel():
    nc = bacc.Bacc("TRN2", target_bir_lowering=False, debug=False)

    x_d = nc.declare_dram_parameter("x", [BPC, C, N], F32, isOutput=False)
    # [cc][128 chans][th th ph ph] and [cc][128 chans][g]
    wq_d = nc.declare_dram_parameter("wq", [2, 128, 128], F32, isOutput=False)
    wg_d = nc.declare_dram_parameter("wg", [2, 128, C2], F32, isOutput=False)
    wo_d = nc.declare_dram_parameter("wo", [C2, C], F32, isOutput=False)  # (gamma*w_o).T
    id_d = nc.declare_dram_parameter("ident", [128, 128], F32, isOutput=False)
    out_d = nc.declare_dram_parameter("out", [BPC, C, N], F32, isOutput=True)

    with tile.TileContext(nc) as tc:
        with (
            tc.tile_pool(name="const", bufs=1) as constp,
            tc.tile_pool(name="xbf", bufs=4) as xbfp,
            tc.tile_pool(name="xres", bufs=6) as xrp,
            tc.tile_pool(name="proj", bufs=2) as projp,
            tc.tile_pool(name="exp", bufs=1) as expp,
            tc.tile_pool(name="gt", bufs=1) as gtp,
            tc.tile_pool(name="small", bufs=3) as smallp,
            tc.tile_pool(name="outs", bufs=4) as outp,
            tc.tile_pool(name="ps_big", bufs=3, space="PSUM") as psb,
            tc.tile_pool(name="ps_a", bufs=1, space="PSUM") as psa,
            tc.tile_pool(name="ps_d", bufs=1, space="PSUM") as psd,
        ):
            # ---- constants / weights (loaded once, cast by DMA) ----
            wq, wg = [], []
            for cc in range(2):
                t = constp.tile([128, 128], BF16, tag=f"wq{cc}")
                nc.gpsimd.dma_start(t[:], wq_d[cc])
                wq.append(t)
                t = constp.tile([128, C2], BF16, tag=f"wg{cc}")
                nc.gpsimd.dma_start(t[:], wg_d[cc])
                wg.append(t)
            wo = constp.tile([C2, C], BF16, tag="wo")
            nc.gpsimd.dma_start(wo[:], wo_d[:])
            id_b = constp.tile([128, 128], BF16, tag="id_b")
            nc.gpsimd.dma_start(id_b[:], id_d[:])
            ones = constp.tile([128, 128], BF16, tag="ones")
            nc.gpsimd.memset(ones[:], 1.0)

            for b in range(BPC):
                # ---- load x as bf16 (casting DMA on gpsimd SWDGE) ----
                xbs = []
                for cc in range(2):
                    xb = xbfp.tile([128, N], BF16, tag="xb", name=f"xb{b}_{cc}")
                    nc.gpsimd.dma_start(xb[:], x_d[b, cc * 128:(cc + 1) * 128, :])
                    xbs.append(xb)

                # ---- projections ----
                thph = projp.tile([128, N], BF16, tag="thph")  # 0:64 dup-theta, 64:128 dup-phi
                g_sb = projp.tile([C2, N], BF16, tag="g_sb")
                for i in range(NNCH):
                    sl = slice(i * NCH, (i + 1) * NCH)
                    ps1 = psb.tile([128, NCH], F32, tag="big", name=f"ps1_{b}_{i}")
                    for cc in range(2):
                        nc.tensor.matmul(ps1[:], wq[cc][:], xbs[cc][:, sl],
                                         start=(cc == 0), stop=(cc == 1))
                    nc.scalar.copy(thph[:, sl], ps1[:])
                    ps2 = psb.tile([128, NCH], F32, tag="big", name=f"ps2_{b}_{i}")
                    for cc in range(2):
                        nc.tensor.matmul(ps2[:], wg[cc][:], xbs[cc][:, sl],
                                         start=(cc == 0), stop=(cc == 1))
                    nc.scalar.copy(g_sb[:, sl], ps2[:])
                th2 = thph[0:64]

                # ---- 2x2 maxpool (w-pairs then h-pairs, strided SBUF ops) ----
                ph2t = projp.tile([64, N // 2], BF16, tag="ph2t")
                pv = thph[:].rearrange("p (w2 two) -> p w2 two", two=2)
                nc.vector.tensor_max(ph2t[:], pv[64:128, :, 0], pv[64:128, :, 1])
                ph2 = projp.tile([64, M], BF16, tag="ph2")
                v2 = ph2t[:].rearrange("p (h2 hb w2) -> p h2 w2 hb", h2=H // 2, hb=2, w2=W // 2)
                nc.vector.tensor_max(ph2[:], v2[:, :, :, 0], v2[:, :, :, 1])
                g_t = projp.tile([C2, N // 2], BF16, tag="g_t")
                pv2 = g_sb[:].rearrange("p (w2 two) -> p w2 two", two=2)
                nc.vector.tensor_max(g_t[:], pv2[:, :, 0], pv2[:, :, 1])
                gp = projp.tile([C2, M], BF16, tag="g_p")
                v2 = g_t[:].rearrange("p (h2 hb w2) -> p h2 w2 hb", h2=H // 2, hb=2, w2=W // 2)
                nc.vector.tensor_max(gp[:], v2[:, :, :, 0], v2[:, :, :, 1])

                # ---- gT: transpose pooled g into 8 [128m, 128c] chunks ----
                gts = []
                for mc in range(MC):
                    tp = psa.tile([128, 128], BF16, tag="a", name=f"tp{b}_{mc}")
                    nc.tensor.transpose(tp[:], gp[:, mc * 128:(mc + 1) * 128], id_b[:])
                    gt = gtp.tile([128, 128], BF16, tag=f"gt{mc}", name=f"gt{mc}_{b}")
                    nc.vector.tensor_copy(gt[:], tp[:])
                    gts.append(gt)

                # ---- scores/exp interleaved with attention at round granularity ----
                # PE executes its stream in order, so alternate 4 score MMs
                # (one round) with 8 attention-accumulation MMs; ScalarE exps
                # overlap the attention matmuls.
                ets = []
                for mc in range(MC):
                    et = expp.tile([128, N], BF16, tag=f"expT{mc}", name=f"expT{mc}_{b}")
                    ets.append(et)

                # deferred attention work-units, 8 accumulation MMs each:
                # (chunk, 'attn') and (chunk, 'den') + epilogue after 'den'
                aps_map = {}

                def unit_attn(i):
                    nsl = slice(i * NCH, (i + 1) * NCH)
                    aps = psa.tile([128, NCH], F32, tag="a", name=f"aps{b}_{i}")
                    aps_map[i] = aps
                    for mc in range(MC):
                        nc.tensor.matmul(aps[:], gts[mc][:], ets[mc][:, nsl],
                                         start=(mc == 0), stop=(mc == MC - 1),
                                         skip_group_check=True)

                def unit_den_epi(i):
                    nsl = slice(i * NCH, (i + 1) * NCH)
                    aps = aps_map.pop(i)
                    dps = psd.tile([128, NCH], F32, tag="d", name=f"dps{b}_{i}")
                    xr = xrp.tile([128, 1024], F32, tag="xr", name=f"xr{b}_{i}")
                    for oc in range(2):
                        nc.sync.dma_start(xr[:, oc * NCH:(oc + 1) * NCH],
                                          x_d[b, oc * 128:(oc + 1) * 128, nsl])
                    for mc in range(MC):
                        nc.tensor.matmul(dps[:], ones[:], ets[mc][:, nsl],
                                         start=(mc == 0), stop=(mc == MC - 1),
                                         skip_group_check=True)
                    rec = smallp.tile([128, NCH], F32, tag="rec", name=f"rec{b}_{i}")
                    nc.vector.reciprocal_approx_fast(rec[:], dps[:])
                    at = smallp.tile([128, NCH], BF16, tag="attn", name=f"at{b}_{i}")
                    nc.vector.scalar_tensor_tensor(
                        at[:], aps[:], 1.0, rec[:],
                        mybir.AluOpType.bypass, mybir.AluOpType.mult)
                    op0 = psa.tile([128, NCH], F32, tag="a", name=f"op0_{b}_{i}")
                    nc.tensor.matmul(op0[:], wo[:, 0:128], at[:], start=True, stop=True)
                    op1 = psd.tile([128, NCH], F32, tag="d", name=f"op1_{b}_{i}")
                    nc.tensor.matmul(op1[:], wo[:, 128:256], at[:], start=True, stop=True)
                    for oc, ops in ((0, op0), (1, op1)):
                        osb = outp.tile([128, NCH], F32, tag="osb",
                                        name=f"osb{b}_{i}_{oc}")
                        nc.vector.scalar_tensor_tensor(
                            osb[:], ops[:], 1.0, xr[:, oc * NCH:(oc + 1) * NCH],
                            mybir.AluOpType.bypass, mybir.AluOpType.add)
                        nc.sync.dma_start(out_d[b, oc * 128:(oc + 1) * 128, nsl],
                                          osb[:])

                units = []
                for i in range(NNCH):
                    units.append(lambda i=i: unit_attn(i))
                    units.append(lambda i=i: unit_den_epi(i))
                uidx = 0

                for qt in range(5):
                    if qt < 4:
                        qsl = slice(qt * 1024, (qt + 1) * 1024)
                        for r in range(4):
                            mc_a, mc_b = 2 * r, 2 * r + 1
                            spa = psb.tile([128, 1024], F32, tag="big",
                                           name=f"spa{b}_{qt}_{r}")
                            spb = psb.tile([128, 1024], F32, tag="big",
                                           name=f"spb{b}_{qt}_{r}")
                            for hf in range(2):
                                nsl = slice(qt * 1024 + hf * 512, qt * 1024 + (hf + 1) * 512)
                                osl = slice(hf * 512, (hf + 1) * 512)
                                nc.tensor.matmul(
                                    spa[:, osl], ph2[0:32, mc_a * 128:(mc_a + 1) * 128],
                                    th2[0:32, nsl], start=True, stop=True)
                                nc.tensor.matmul(
                                    spb[:, osl], ph2[32:64, mc_b * 128:(mc_b + 1) * 128],
                                    th2[32:64, nsl], start=True, stop=True)
                            nc.scalar.activation(ets[mc_a][:, qsl], spa[:],
                                                 mybir.ActivationFunctionType.Exp)
                            nc.scalar.activation(ets[mc_b][:, qsl], spb[:],
                                                 mybir.ActivationFunctionType.Exp)
                            if qt >= 1 and uidx < len(units):
                                units[uidx](); uidx += 1
                    else:
                        while uidx < len(units):
                            units[uidx](); uidx += 1

    nc.compile()
    return nc


_NC_CACHE = None


def _get_nc():
    global _NC_CACHE
    if _NC_CACHE is None:
        _NC_CACHE = build_kernel()
    return _NC_CACHE


def prep_inputs(x, w_theta, w_phi, w_g, w_o, gamma):
    """Host-side prep: shard x over 8 cores; transpose/scale/pack weights."""
    x = np.asarray(x, dtype=np.float32).reshape(B, C, N)
    w_theta = np.asarray(w_theta, dtype=np.float32)
    w_phi = np.asarray(w_phi, dtype=np.float32)
    w_g = np.asarray(w_g, dtype=np.float32)
    w_o = np.asarray(w_o, dtype=np.float32)
    gamma = np.float32(gamma)

    # combined projection weight: [th th ph ph] along output dim
    wqT = np.concatenate([w_theta.T, w_theta.T, w_phi.T, w_phi.T], axis=1)  # [256,128]
    wq = np.ascontiguousarray(wqT.reshape(2, 128, 128))
    wgq = np.ascontiguousarray(w_g.T.reshape(2, 128, C2))
    wo = np.ascontiguousarray((gamma * w_o).T)
    ident = np.eye(128, dtype=np.float32)

    in_maps = []
    for core in range(NCORES):
        shard = np.ascontiguousarray(x[core * BPC:(core + 1) * BPC])
        in_maps.append({"x": shard, "wq": wq, "wg": wgq, "wo": wo, "ident": ident})
    return in_maps


def run(inputs, trace=False, **kw):
    nc = _get_nc()
    in_maps = prep_inputs(**inputs)
    res = run_bass_kernel_spmd(nc, in_maps, core_ids=list(range(NCORES)),
                               trace=trace, **kw)
    outs = [res.results[i]["out"] for i in range(NCORES)]
    full = np.concatenate(outs, axis=0).reshape(B, C, H, W).astype(np.float32)
    return full, res


def kernel(**inputs):
    full, _ = run(inputs, trace=False)
    return full


# revision 25
# speedup vs baseline: 1.2784x; 1.0682x over previous
"""Self-attention (SAGAN-style) Trainium2 kernel.

Reference computation (per batch sample):
    theta = w_theta @ x            # [32, 4096]
    phi   = pool2x2(w_phi @ x)     # [32, 1024]
    g     = pool2x2(w_g @ x)       # [128, 1024]
    beta  = softmax(theta.T @ phi, axis=-1)   # [4096, 1024]
    attn  = g @ beta.T             # [128, 4096]
    out   = gamma * (w_o @ attn) + x

Sharding: data-parallel over batch; B=16 over 8 cores -> 2 samples/core.

Kernel strategy (per core, per sample), all matmuls bf16 (fp32 PSUM accum):
  - x loaded via gpsimd casting DMA straight to bf16; the fp32 x needed for
    the residual is re-DMAed in [128,512] chunks at consume time.
  - one combined projection weight [256, 128] computes theta twice and phi
    twice (rows 0:32/32:64 theta, 64:96/96:128 phi) so the K=32 score matmuls
    can run 2-way row-tiled (tile_position (0,0)/(32,0)).
  - scoresT in [m, n] layout; exp on ScalarE straight out of PSUM -> bf16
    SBUF (logits are O(+-40): exp without max-subtraction is safe). The
    score/exp work for quarter qt is emitted together with the attention
    for quarter qt-1 so the PE has matmul work while ScalarE exps.
  - attn[c, n] = sum_mc gT[mc].T @ expT[mc]; gT from PE transposes of pooled
    g. The softmax denominator rides the same rhs streams through an all-ones
    stationary operand, which also broadcasts the row-sum to all partitions.
  - normalize via reciprocal_approx_fast + scalar_tensor_tensor;
    o = (gamma*w_o).T @ attn; residual fused into PSUM evacuation.
"""

import numpy as np

import concourse.bacc as bacc
import concourse.mybir as mybir
from concourse import tile
from concourse.bass_utils import run_bass_kernel_spmd

F32 = mybir.dt.float32
BF16 = mybir.dt.bfloat16

B, C, H, W = 16, 256, 64, 64
N = H * W            # 4096
M = N // 4           # 1024
C8 = C // 8          # 32
C2 = C // 2          # 128
NCORES = 8
BPC = B // NCORES    # 2 samples per core
NCH = 512            # n-chunk width for matmul streaming
NNCH = N // NCH      # 8
MC = M // 128        # 8 m-chunks


def build_kernel():
    nc = bacc.Bacc("TRN2", target_bir_lowering=False, debug=False)

    x_d = nc.declare_dram_parameter("x", [BPC, C, N], F32, isOutput=False)
    # [cc][128 chans][th th ph ph] and [cc][128 chans][g]
    wq_d = nc.declare_dram_parameter("wq", [2, 128, 128], F32, isOutput=False)
    wg_d = nc.declare_dram_parameter("wg", [2, 128, C2], F32, isOutput=False)
    wo_d = nc.declare_dram_parameter("wo", [C2, C], F32, isOutput=False)  # (gamma*w_o).T
    id_d = nc.declare_dram_parameter("ident", [128, 128], F32, isOutput=False)
    out_d = nc.declare_dram_parameter("out", [BPC, C, N], F32, isOutput=True)

    with tile.TileContext(nc) as tc:
        with (
            tc.tile_pool(name="const", bufs=1) as constp,
            tc.tile_pool(name="xbf", bufs=4) as xbfp,
            tc.tile_pool(name="xres", bufs=6) as xrp,
            tc.tile_pool(name="proj", bufs=2) as projp,
            tc.tile_pool(name="exp", bufs=1) as expp,
            tc.tile_pool(name="gt", bufs=1) as gtp,
            tc.tile_pool(name="small", bufs=3) as smallp,
            tc.tile_pool(name="outs", bufs=4) as outp,
            tc.tile_pool(name="ps_big", bufs=3, space="PSUM") as psb,
            tc.tile_pool(name="ps_a", bufs=1, space="PSUM") as psa,
            tc.tile_pool(name="ps_d", bufs=1, space="PSUM") as psd,
        ):
            # ---- constants / weights (loaded once, cast by DMA) ----
            wq, wg = [], []
            for cc in range(2):
                t = constp.tile([128, 128], BF16, tag=f"wq{cc}")
                nc.gpsimd.dma_start(t[:], wq_d[cc])
                wq.append(t)
                t = constp.tile([128, C2], BF16, tag=f"wg{cc}")
                nc.gpsimd.dma_start(t[:], wg_d[cc])
                wg.append(t)
            wo = constp.tile([C2, C], BF16, tag="wo")
            nc.gpsimd.dma_start(wo[:], wo_d[:])
            id_b = constp.tile([128, 128], BF16, tag="id_b")
            nc.gpsimd.dma_start(id_b[:], id_d[:])
            ones = constp.tile([128, 128], BF16, tag="ones")
            nc.gpsimd.memset(ones[:], 1.0)

            for b in range(BPC):
                # ---- load x as bf16 (casting DMA on gpsimd SWDGE) ----
                xbs = []
                for cc in range(2):
                    xb = xbfp.tile([128, N], BF16, tag="xb", name=f"xb{b}_{cc}")
                    nc.gpsimd.dma_start(xb[:], x_d[b, cc * 128:(cc + 1) * 128, :])
                    xbs.append(xb)

                # ---- projections ----
                thph = projp.tile([128, N], BF16, tag="thph")  # 0:64 dup-theta, 64:128 dup-phi
                g_sb = projp.tile([C2, N], BF16, tag="g_sb")
                for i in range(NNCH):
                    sl = slice(i * NCH, (i + 1) * NCH)
                    ps1 = psb.tile([128, NCH], F32, tag="big", name=f"ps1_{b}_{i}")
                    for cc in range(2):
                        nc.tensor.matmul(ps1[:], wq[cc][:], xbs[cc][:, sl],
                                         start=(cc == 0), stop=(cc == 1))
                    nc.scalar.copy(thph[:, sl], ps1[:])
                    ps2 = psb.tile([128, NCH], F32, tag="big", name=f"ps2_{b}_{i}")
                    for cc in range(2):
                        nc.tensor.matmul(ps2[:], wg[cc][:], xbs[cc][:, sl],
                                         start=(cc == 0), stop=(cc == 1))
                    nc.scalar.copy(g_sb[:, sl], ps2[:])
                    # fused w-pair maxpool for this 512-column chunk
                    if i == 0:
                        ph2t = projp.tile([64, N // 2], BF16, tag="ph2t")
                        g_t = projp.tile([C2, N // 2], BF16, tag="g_t")
                    csl = slice(i * 256, (i + 1) * 256)
                    pv = thph[:].rearrange("p (w2 two) -> p w2 two", two=2)
                    nc.vector.tensor_max(ph2t[:, csl], pv[64:128, csl, 0], pv[64:128, csl, 1])
                    pv2 = g_sb[:].rearrange("p (w2 two) -> p w2 two", two=2)
                    nc.vector.tensor_max(g_t[:, csl], pv2[:, csl, 0], pv2[:, csl, 1])
                th2 = thph[0:64]

                # ---- h-pair maxpool, chunked per mc so consumers start early ----
                ph2 = projp.tile([64, M], BF16, tag="ph2")
                gp = projp.tile([C2, M], BF16, tag="g_p")
                vph = ph2t[:].rearrange("p (h2 hb w2) -> p h2 w2 hb", h2=H // 2, hb=2, w2=W // 2)
                vg = g_t[:].rearrange("p (h2 hb w2) -> p h2 w2 hb", h2=H // 2, hb=2, w2=W // 2)
                for mc in range(MC):
                    h4 = slice(4 * mc, 4 * (mc + 1))
                    msl = slice(mc * 128, (mc + 1) * 128)
                    nc.vector.tensor_max(ph2[:, msl], vph[:, h4, :, 0], vph[:, h4, :, 1])
                    nc.vector.tensor_max(gp[:, msl], vg[:, h4, :, 0], vg[:, h4, :, 1])

                # gT transposes are emitted inside quarter 0 as PE fillers
                gts = []

                def emit_transpose(mc):
                    tp = psa.tile([128, 128], BF16, tag="a", name=f"tp{b}_{mc}")
                    nc.tensor.transpose(tp[:], gp[:, mc * 128:(mc + 1) * 128], id_b[:])
                    gt = gtp.tile([128, 128], BF16, tag=f"gt{mc}", name=f"gt{mc}_{b}")
                    nc.vector.tensor_copy(gt[:], tp[:])
                    gts.append(gt)

                # ---- scores/exp interleaved with attention at round granularity ----
                # PE executes its stream in order, so alternate 4 score MMs
                # (one round) with 8 attention-accumulation MMs; ScalarE exps
                # overlap the attention matmuls.
                ets = []
                for mc in range(MC):
                    et = expp.tile([128, N], BF16, tag=f"expT{mc}", name=f"expT{mc}_{b}")
                    ets.append(et)

                # deferred attention work-units, 8 accumulation MMs each:
                # (chunk, 'attn') and (chunk, 'den') + epilogue after 'den'
                aps_map = {}

                def unit_attn(i):
                    nsl = slice(i * NCH, (i + 1) * NCH)
                    aps = psa.tile([128, NCH], F32, tag="a", name=f"aps{b}_{i}")
                    aps_map[i] = aps
                    for mc in range(MC):
                        nc.tensor.matmul(aps[:], gts[mc][:], ets[mc][:, nsl],
                                         start=(mc == 0), stop=(mc == MC - 1),
                                         skip_group_check=True)

                def unit_den_epi(i):
                    nsl = slice(i * NCH, (i + 1) * NCH)
                    aps = aps_map.pop(i)
                    dps = psd.tile([128, NCH], F32, tag="d", name=f"dps{b}_{i}")
                    xr = xrp.tile([128, 1024], F32, tag="xr", name=f"xr{b}_{i}")
                    for oc in range(2):
                        nc.sync.dma_start(xr[:, oc * NCH:(oc + 1) * NCH],
                                          x_d[b, oc * 128:(oc + 1) * 128, nsl])
                    for mc in range(MC):
                        nc.tensor.matmul(dps[:], ones[:], ets[mc][:, nsl],
                                         start=(mc == 0), stop=(mc == MC - 1),
                                         skip_group_check=True)
                    rec = smallp.tile([128, NCH], F32, tag="rec", name=f"rec{b}_{i}")
                    nc.vector.reciprocal_approx_fast(rec[:], dps[:])
                    at = smallp.tile([128, NCH], BF16, tag="attn", name=f"at{b}_{i}")
                    nc.vector.scalar_tensor_tensor(
                        at[:], aps[:], 1.0, rec[:],
                        mybir.AluOpType.bypass, mybir.AluOpType.mult)
                    op0 = psa.tile([128, NCH], F32, tag="a", name=f"op0_{b}_{i}")
                    nc.tensor.matmul(op0[:], wo[:, 0:128], at[:], start=True, stop=True)
                    op1 = psd.tile([128, NCH], F32, tag="d", name=f"op1_{b}_{i}")
                    nc.tensor.matmul(op1[:], wo[:, 128:256], at[:], start=True, stop=True)
                    for oc, ops in ((0, op0), (1, op1)):
                        osb = outp.tile([128, NCH], F32, tag="osb",
                                        name=f"osb{b}_{i}_{oc}")
                        nc.vector.scalar_tensor_tensor(
                            osb[:], ops[:], 1.0, xr[:, oc * NCH:(oc + 1) * NCH],
                            mybir.AluOpType.bypass, mybir.AluOpType.add)
                        nc.sync.dma_start(out_d[b, oc * 128:(oc + 1) * 128, nsl],
                                          osb[:])

                units = []
                for i in range(NNCH):
                    units.append(lambda i=i: unit_attn(i))
                    units.append(lambda i=i: unit_den_epi(i))
                uidx = 0

                for qt in range(5):
                    if qt < 4:
                        qsl = slice(qt * 1024, (qt + 1) * 1024)
                        for r in range(4):
                            mc_a, mc_b = 2 * r, 2 * r + 1
                            spa = psb.tile([128, 1024], F32, tag="big",
                                           name=f"spa{b}_{qt}_{r}")
                            spb = psb.tile([128, 1024], F32, tag="big",
                                           name=f"spb{b}_{qt}_{r}")
                            for hf in range(2):
                                nsl = slice(qt * 1024 + hf * 512, qt * 1024 + (hf + 1) * 512)
                                osl = slice(hf * 512, (hf + 1) * 512)
                                nc.tensor.matmul(
                                    spa[:, osl], ph2[0:32, mc_a * 128:(mc_a + 1) * 128],
                                    th2[0:32, nsl], start=True, stop=True)
                                nc.tensor.matmul(
                                    spb[:, osl], ph2[32:64, mc_b * 128:(mc_b + 1) * 128],
                                    th2[32:64, nsl], start=True, stop=True)
                            nc.scalar.activation(ets[mc_a][:, qsl], spa[:],
                                                 mybir.ActivationFunctionType.Exp)
                            nc.scalar.activation(ets[mc_b][:, qsl], spb[:],
                                                 mybir.ActivationFunctionType.Exp)
                            if qt == 0:
                                emit_transpose(2 * r)
                                emit_transpose(2 * r + 1)
                            elif uidx < len(units):
                                units[uidx](); uidx += 1
                    else:
                        while uidx < len(units):
                            units[uidx](); uidx += 1

    nc.compile()
    return nc


_NC_CACHE = None


def _get_nc():
    global _NC_CACHE
    if _NC_CACHE is None:
        _NC_CACHE = build_kernel()
    return _NC_CACHE


def prep_inputs(x, w_theta, w_phi, w_g, w_o, gamma):
    """Host-side prep: shard x over 8 cores; transpose/scale/pack weights."""
    x = np.asarray(x, dtype=np.float32).reshape(B, C, N)
    w_theta = np.asarray(w_theta, dtype=np.float32)
    w_phi = np.asarray(w_phi, dtype=np.float32)
    w_g = np.asarray(w_g, dtype=np.float32)
    w_o = np.asarray(w_o, dtype=np.float32)
    gamma = np.float32(gamma)

    # combined projection weight: [th th ph ph] along output dim
    wqT = np.concatenate([w_theta.T, w_theta.T, w_phi.T, w_phi.T], axis=1)  # [256,128]
    wq = np.ascontiguousarray(wqT.reshape(2, 128, 128))
    wgq = np.ascontiguousarray(w_g.T.reshape(2, 128, C2))
    wo = np.ascontiguousarray((gamma * w_o).T)
    ident = np.eye(128, dtype=np.float32)

    in_maps = []
    for core in range(NCORES):
        shard = np.ascontiguousarray(x[core * BPC:(core + 1) * BPC])
        in_maps.append({"x": shard, "wq": wq, "wg": wgq, "wo": wo, "ident": ident})
    return in_maps


def run(inputs, trace=False, **kw):
    nc = _get_nc()
    in_maps = prep_inputs(**inputs)
    res = run_bass_kernel_spmd(nc, in_maps, core_ids=list(range(NCORES)),
                               trace=trace, **kw)
    outs = [res.results[i]["out"] for i in range(NCORES)]
    full = np.concatenate(outs, axis=0).reshape(B, C, H, W).astype(np.float32)
    return full, res


def kernel(**inputs):
    full, _ = run(inputs, trace=False)
    return full


# revision 29
# speedup vs baseline: 1.3123x; 1.0265x over previous
"""Self-attention (SAGAN-style) Trainium2 kernel.

Reference computation (per batch sample):
    theta = w_theta @ x            # [32, 4096]
    phi   = pool2x2(w_phi @ x)     # [32, 1024]
    g     = pool2x2(w_g @ x)       # [128, 1024]
    beta  = softmax(theta.T @ phi, axis=-1)   # [4096, 1024]
    attn  = g @ beta.T             # [128, 4096]
    out   = gamma * (w_o @ attn) + x

Sharding: data-parallel over batch; B=16 over 8 cores -> 2 samples/core.

Kernel strategy (per core, per sample), all matmuls bf16 (fp32 PSUM accum):
  - x loaded via gpsimd casting DMA straight to bf16; the fp32 x needed for
    the residual is re-DMAed in [128,512] chunks at consume time.
  - one combined projection weight [256, 128] computes theta twice and phi
    twice (rows 0:32/32:64 theta, 64:96/96:128 phi) so the K=32 score matmuls
    can run 2-way row-tiled (tile_position (0,0)/(32,0)).
  - scoresT in [m, n] layout; exp on ScalarE straight out of PSUM -> bf16
    SBUF (logits are O(+-40): exp without max-subtraction is safe). The
    score/exp work for quarter qt is emitted together with the attention
    for quarter qt-1 so the PE has matmul work while ScalarE exps.
  - attn[c, n] = sum_mc gT[mc].T @ expT[mc]; gT from PE transposes of pooled
    g. The softmax denominator rides the same rhs streams through an all-ones
    stationary operand, which also broadcasts the row-sum to all partitions.
  - normalize via reciprocal_approx_fast + scalar_tensor_tensor;
    o = (gamma*w_o).T @ attn; residual fused into PSUM evacuation.
"""

import numpy as np

import concourse.bacc as bacc
import concourse.mybir as mybir
from concourse import tile
from concourse.bass_utils import run_bass_kernel_spmd

F32 = mybir.dt.float32
BF16 = mybir.dt.bfloat16

B, C, H, W = 16, 256, 64, 64
N = H * W            # 4096
M = N // 4           # 1024
C8 = C // 8          # 32
C2 = C // 2          # 128
NCORES = 8
BPC = B // NCORES    # 2 samples per core
NCH = 512            # n-chunk width for matmul streaming
NNCH = N // NCH      # 8
MC = M // 128        # 8 m-chunks


def build_kernel():
    nc = bacc.Bacc("TRN2", target_bir_lowering=False, debug=False)

    x_d = nc.declare_dram_parameter("x", [BPC, C, N], F32, isOutput=False)
    # [cc][128 chans][th th ph ph] and [cc][128 chans][g]
    wq_d = nc.declare_dram_parameter("wq", [2, 128, 128], F32, isOutput=False)
    wg_d = nc.declare_dram_parameter("wg", [2, 128, C2], F32, isOutput=False)
    wo_d = nc.declare_dram_parameter("wo", [C2, C], F32, isOutput=False)  # (gamma*w_o).T
    id_d = nc.declare_dram_parameter("ident", [128, 128], F32, isOutput=False)
    out_d = nc.declare_dram_parameter("out", [BPC, C, N], F32, isOutput=True)

    with tile.TileContext(nc) as tc:
        with (
            tc.tile_pool(name="const", bufs=1) as constp,
            tc.tile_pool(name="xbf", bufs=4) as xbfp,
            tc.tile_pool(name="xres", bufs=6) as xrp,
            tc.tile_pool(name="proj", bufs=2) as projp,
            tc.tile_pool(name="exp", bufs=1) as expp,
            tc.tile_pool(name="gt", bufs=1) as gtp,
            tc.tile_pool(name="small", bufs=3) as smallp,
            tc.tile_pool(name="outs", bufs=4) as outp,
            tc.tile_pool(name="ps_big", bufs=3, space="PSUM") as psb,
            tc.tile_pool(name="ps_a", bufs=1, space="PSUM") as psa,
            tc.tile_pool(name="ps_d", bufs=1, space="PSUM") as psd,
        ):
            # ---- constants / weights (loaded once, cast by DMA) ----
            wq, wg = [], []
            for cc in range(2):
                t = constp.tile([128, 128], BF16, tag=f"wq{cc}")
                nc.gpsimd.dma_start(t[:], wq_d[cc])
                wq.append(t)
                t = constp.tile([128, C2], BF16, tag=f"wg{cc}")
                nc.gpsimd.dma_start(t[:], wg_d[cc])
                wg.append(t)
            wo = constp.tile([C2, C], BF16, tag="wo")
            nc.gpsimd.dma_start(wo[:], wo_d[:])
            id_b = constp.tile([128, 128], BF16, tag="id_b")
            nc.gpsimd.dma_start(id_b[:], id_d[:])
            ones = constp.tile([128, 128], BF16, tag="ones")
            nc.gpsimd.memset(ones[:], 1.0)

            pending = []

            def pop_unit():
                if pending:
                    pending.pop(0)()

            for b in range(BPC):
                # ---- load x as bf16 (casting DMA on gpsimd SWDGE) ----
                xbs = []
                for cc in range(2):
                    xb = xbfp.tile([128, N], BF16, tag="xb", name=f"xb{b}_{cc}")
                    nc.gpsimd.dma_start(xb[:], x_d[b, cc * 128:(cc + 1) * 128, :])
                    xbs.append(xb)

                # ---- projections ----
                thph = projp.tile([128, N], BF16, tag="thph")  # 0:64 dup-theta, 64:128 dup-phi
                g_sb = projp.tile([C2, N], BF16, tag="g_sb")
                for i in range(NNCH):
                    sl = slice(i * NCH, (i + 1) * NCH)
                    ps1 = psb.tile([128, NCH], F32, tag="big", name=f"ps1_{b}_{i}")
                    for cc in range(2):
                        nc.tensor.matmul(ps1[:], wq[cc][:], xbs[cc][:, sl],
                                         start=(cc == 0), stop=(cc == 1))
                    nc.scalar.copy(thph[:, sl], ps1[:])
                    ps2 = psb.tile([128, NCH], F32, tag="big", name=f"ps2_{b}_{i}")
                    for cc in range(2):
                        nc.tensor.matmul(ps2[:], wg[cc][:], xbs[cc][:, sl],
                                         start=(cc == 0), stop=(cc == 1))
                    nc.scalar.copy(g_sb[:, sl], ps2[:])
                    pop_unit()
                    # fused w-pair maxpool for this 512-column chunk
                    if i == 0:
                        ph2t = projp.tile([64, N // 2], BF16, tag="ph2t")
                        g_t = projp.tile([C2, N // 2], BF16, tag="g_t")
                    csl = slice(i * 256, (i + 1) * 256)
                    pv = thph[:].rearrange("p (w2 two) -> p w2 two", two=2)
                    nc.vector.tensor_max(ph2t[:, csl], pv[64:128, csl, 0], pv[64:128, csl, 1])
                    pv2 = g_sb[:].rearrange("p (w2 two) -> p w2 two", two=2)
                    nc.vector.tensor_max(g_t[:, csl], pv2[:, csl, 0], pv2[:, csl, 1])
                th2 = thph[0:64]

                # ---- h-pair maxpool, chunked per mc so consumers start early ----
                ph2 = projp.tile([64, M], BF16, tag="ph2")
                gp = projp.tile([C2, M], BF16, tag="g_p")
                vph = ph2t[:].rearrange("p (h2 hb w2) -> p h2 w2 hb", h2=H // 2, hb=2, w2=W // 2)
                vg = g_t[:].rearrange("p (h2 hb w2) -> p h2 w2 hb", h2=H // 2, hb=2, w2=W // 2)
                for mc in range(MC):
                    h4 = slice(4 * mc, 4 * (mc + 1))
                    msl = slice(mc * 128, (mc + 1) * 128)
                    nc.vector.tensor_max(ph2[:, msl], vph[:, h4, :, 0], vph[:, h4, :, 1])
                    nc.vector.tensor_max(gp[:, msl], vg[:, h4, :, 0], vg[:, h4, :, 1])

                # gT transposes are emitted inside quarter 0 as PE fillers
                gts = []

                def emit_transpose(mc):
                    tp = psa.tile([128, 128], BF16, tag="a", name=f"tp{b}_{mc}")
                    nc.tensor.transpose(tp[:], gp[:, mc * 128:(mc + 1) * 128], id_b[:])
                    gt = gtp.tile([128, 128], BF16, tag=f"gt{mc}", name=f"gt{mc}_{b}")
                    nc.vector.tensor_copy(gt[:], tp[:])
                    gts.append(gt)

                # ---- scores/exp interleaved with attention at round granularity ----
                # PE executes its stream in order, so alternate 4 score MMs
                # (one round) with 8 attention-accumulation MMs; ScalarE exps
                # overlap the attention matmuls.
                ets = []
                for mc in range(MC):
                    et = expp.tile([128, N], BF16, tag=f"expT{mc}", name=f"expT{mc}_{b}")
                    ets.append(et)

                # deferred attention work-units, 8 accumulation MMs each:
                # (chunk, 'attn') and (chunk, 'den') + epilogue after 'den'
                aps_map = {}

                def unit_attn(i, b=b, ets=ets, gts=gts, aps_map=aps_map):
                    nsl = slice(i * NCH, (i + 1) * NCH)
                    aps = psa.tile([128, NCH], F32, tag="a", name=f"aps{b}_{i}")
                    aps_map[i] = aps
                    for mc in range(MC):
                        nc.tensor.matmul(aps[:], gts[mc][:], ets[mc][:, nsl],
                                         start=(mc == 0), stop=(mc == MC - 1),
                                         skip_group_check=True)

                def unit_den_epi(i, b=b, ets=ets, gts=gts, aps_map=aps_map):
                    nsl = slice(i * NCH, (i + 1) * NCH)
                    aps = aps_map.pop(i)
                    dps = psd.tile([128, NCH], F32, tag="d", name=f"dps{b}_{i}")
                    xr = xrp.tile([128, 1024], F32, tag="xr", name=f"xr{b}_{i}")
                    for oc in range(2):
                        nc.sync.dma_start(xr[:, oc * NCH:(oc + 1) * NCH],
                                          x_d[b, oc * 128:(oc + 1) * 128, nsl])
                    for mc in range(MC):
                        nc.tensor.matmul(dps[:], ones[:], ets[mc][:, nsl],
                                         start=(mc == 0), stop=(mc == MC - 1),
                                         skip_group_check=True)
                    rec = smallp.tile([128, NCH], F32, tag="rec", name=f"rec{b}_{i}")
                    nc.vector.reciprocal_approx_fast(rec[:], dps[:])
                    at = smallp.tile([128, NCH], BF16, tag="attn", name=f"at{b}_{i}")
                    nc.vector.scalar_tensor_tensor(
                        at[:], aps[:], 1.0, rec[:],
                        mybir.AluOpType.bypass, mybir.AluOpType.mult)
                    op0 = psa.tile([128, NCH], F32, tag="a", name=f"op0_{b}_{i}")
                    nc.tensor.matmul(op0[:], wo[:, 0:128], at[:], start=True, stop=True)
                    op1 = psd.tile([128, NCH], F32, tag="d", name=f"op1_{b}_{i}")
                    nc.tensor.matmul(op1[:], wo[:, 128:256], at[:], start=True, stop=True)
                    for oc, ops in ((0, op0), (1, op1)):
                        osb = outp.tile([128, NCH], F32, tag="osb",
                                        name=f"osb{b}_{i}_{oc}")
                        nc.vector.scalar_tensor_tensor(
                            osb[:], ops[:], 1.0, xr[:, oc * NCH:(oc + 1) * NCH],
                            mybir.AluOpType.bypass, mybir.AluOpType.add)
                        nc.sync.dma_start(out_d[b, oc * 128:(oc + 1) * 128, nsl],
                                          osb[:])

                for i in range(NNCH):
                    pending.append(lambda f=unit_attn, i=i: f(i))
                    pending.append(lambda f=unit_den_epi, i=i: f(i))

                for qt in range(5):
                    if qt < 4:
                        qsl = slice(qt * 1024, (qt + 1) * 1024)
                        for r in range(4):
                            mc_a, mc_b = 2 * r, 2 * r + 1
                            spa = psb.tile([128, 1024], F32, tag="big",
                                           name=f"spa{b}_{qt}_{r}")
                            spb = psb.tile([128, 1024], F32, tag="big",
                                           name=f"spb{b}_{qt}_{r}")
                            for hf in range(2):
                                nsl = slice(qt * 1024 + hf * 512, qt * 1024 + (hf + 1) * 512)
                                osl = slice(hf * 512, (hf + 1) * 512)
                                nc.tensor.matmul(
                                    spa[:, osl], ph2[0:32, mc_a * 128:(mc_a + 1) * 128],
                                    th2[0:32, nsl], start=True, stop=True)
                                nc.tensor.matmul(
                                    spb[:, osl], ph2[32:64, mc_b * 128:(mc_b + 1) * 128],
                                    th2[32:64, nsl], start=True, stop=True)
                            nc.scalar.activation(ets[mc_a][:, qsl], spa[:],
                                                 mybir.ActivationFunctionType.Exp)
                            nc.scalar.activation(ets[mc_b][:, qsl], spb[:],
                                                 mybir.ActivationFunctionType.Exp)
                            if qt == 0:
                                emit_transpose(2 * r)
                                emit_transpose(2 * r + 1)
                            else:
                                pop_unit()
                    else:
                        keep = 4 if b == 0 else 0
                        while len(pending) > keep:
                            pop_unit()

    nc.compile()
    return nc


_NC_CACHE = None


def _get_nc():
    global _NC_CACHE
    if _NC_CACHE is None:
        _NC_CACHE = build_kernel()
    return _NC_CACHE


def prep_inputs(x, w_theta, w_phi, w_g, w_o, gamma):
    """Host-side prep: shard x over 8 cores; transpose/scale/pack weights."""
    x = np.asarray(x, dtype=np.float32).reshape(B, C, N)
    w_theta = np.asarray(w_theta, dtype=np.float32)
    w_phi = np.asarray(w_phi, dtype=np.float32)
    w_g = np.asarray(w_g, dtype=np.float32)
    w_o = np.asarray(w_o, dtype=np.float32)
    gamma = np.float32(gamma)

    # combined projection weight: [th th ph ph] along output dim
    wqT = np.concatenate([w_theta.T, w_theta.T, w_phi.T, w_phi.T], axis=1)  # [256,128]
    wq = np.ascontiguousarray(wqT.reshape(2, 128, 128))
    wgq = np.ascontiguousarray(w_g.T.reshape(2, 128, C2))
    wo = np.ascontiguousarray((gamma * w_o).T)
    ident = np.eye(128, dtype=np.float32)

    in_maps = []
    for core in range(NCORES):
        shard = np.ascontiguousarray(x[core * BPC:(core + 1) * BPC])
        in_maps.append({"x": shard, "wq": wq, "wg": wgq, "wo": wo, "ident": ident})
    return in_maps


def run(inputs, trace=False, **kw):
    nc = _get_nc()
    in_maps = prep_inputs(**inputs)
    res = run_bass_kernel_spmd(nc, in_maps, core_ids=list(range(NCORES)),
                               trace=trace, **kw)
    outs = [res.results[i]["out"] for i in range(NCORES)]
    full = np.concatenate(outs, axis=0).reshape(B, C, H, W).astype(np.float32)
    return full, res


def kernel(**inputs):
    full, _ = run(inputs, trace=False)
    return full


# revision 30
# speedup vs baseline: 1.4142x; 1.0777x over previous
"""Self-attention (SAGAN-style) Trainium2 kernel.

Reference computation (per batch sample):
    theta = w_theta @ x            # [32, 4096]
    phi   = pool2x2(w_phi @ x)     # [32, 1024]
    g     = pool2x2(w_g @ x)       # [128, 1024]
    beta  = softmax(theta.T @ phi, axis=-1)   # [4096, 1024]
    attn  = g @ beta.T             # [128, 4096]
    out   = gamma * (w_o @ attn) + x

Sharding: data-parallel over batch; B=16 over 8 cores -> 2 samples/core.

Kernel strategy (per core, per sample), all matmuls bf16 (fp32 PSUM accum):
  - x loaded via gpsimd casting DMA straight to bf16; the fp32 x needed for
    the residual is re-DMAed in [128,512] chunks at consume time.
  - one combined projection weight [256, 128] computes theta twice and phi
    twice (rows 0:32/32:64 theta, 64:96/96:128 phi) so the K=32 score matmuls
    can run 2-way row-tiled (tile_position (0,0)/(32,0)).
  - scoresT in [m, n] layout; exp on ScalarE straight out of PSUM -> bf16
    SBUF (logits are O(+-40): exp without max-subtraction is safe). The
    score/exp work for quarter qt is emitted together with the attention
    for quarter qt-1 so the PE has matmul work while ScalarE exps.
  - attn[c, n] = sum_mc gT[mc].T @ expT[mc]; gT from PE transposes of pooled
    g. The softmax denominator rides the same rhs streams through an all-ones
    stationary operand, which also broadcasts the row-sum to all partitions.
  - normalize via reciprocal_approx_fast + scalar_tensor_tensor;
    o = (gamma*w_o).T @ attn; residual fused into PSUM evacuation.
"""

import numpy as np

import concourse.bacc as bacc
import concourse.mybir as mybir
from concourse import tile
from concourse.bass_utils import run_bass_kernel_spmd

F32 = mybir.dt.float32
BF16 = mybir.dt.bfloat16

B, C, H, W = 16, 256, 64, 64
N = H * W            # 4096
M = N // 4           # 1024
C8 = C // 8          # 32
C2 = C // 2          # 128
NCORES = 8
BPC = B // NCORES    # 2 samples per core
NCH = 512            # n-chunk width for matmul streaming
NNCH = N // NCH      # 8
MC = M // 128        # 8 m-chunks


def build_kernel():
    nc = bacc.Bacc("TRN2", target_bir_lowering=False, debug=False)

    x_d = nc.declare_dram_parameter("x", [BPC, C, N], F32, isOutput=False)
    # [cc][128 chans][th th ph ph] and [cc][128 chans][g]
    wq_d = nc.declare_dram_parameter("wq", [2, 128, 128], F32, isOutput=False)
    wg_d = nc.declare_dram_parameter("wg", [2, 128, C2], F32, isOutput=False)
    wo_d = nc.declare_dram_parameter("wo", [C2, C], F32, isOutput=False)  # (gamma*w_o).T
    id_d = nc.declare_dram_parameter("ident", [128, 128], F32, isOutput=False)
    out_d = nc.declare_dram_parameter("out", [BPC, C, N], F32, isOutput=True)

    with tile.TileContext(nc) as tc:
        with (
            tc.tile_pool(name="const", bufs=1) as constp,
            tc.tile_pool(name="xbf", bufs=4) as xbfp,
            tc.tile_pool(name="xres", bufs=6) as xrp,
            tc.tile_pool(name="proj", bufs=2) as projp,
            tc.tile_pool(name="exp", bufs=1) as expp,
            tc.tile_pool(name="gt", bufs=1) as gtp,
            tc.tile_pool(name="small", bufs=3) as smallp,
            tc.tile_pool(name="outs", bufs=4) as outp,
            tc.tile_pool(name="ps_big", bufs=3, space="PSUM") as psb,
            tc.tile_pool(name="ps_a", bufs=1, space="PSUM") as psa,
            tc.tile_pool(name="ps_d", bufs=1, space="PSUM") as psd,
        ):
            # ---- constants / weights (loaded once, cast by DMA) ----
            wq, wg = [], []
            for cc in range(2):
                t = constp.tile([128, 128], BF16, tag=f"wq{cc}")
                nc.gpsimd.dma_start(t[:], wq_d[cc])
                wq.append(t)
                t = constp.tile([128, C2], BF16, tag=f"wg{cc}")
                nc.gpsimd.dma_start(t[:], wg_d[cc])
                wg.append(t)
            wo = constp.tile([C2, C], BF16, tag="wo")
            nc.gpsimd.dma_start(wo[:], wo_d[:])
            id_b = constp.tile([128, 128], BF16, tag="id_b")
            nc.gpsimd.dma_start(id_b[:], id_d[:])
            ones = constp.tile([128, 128], BF16, tag="ones")
            nc.gpsimd.memset(ones[:], 1.0)

            pending = []

            def pop_unit():
                if pending:
                    pending.pop(0)()

            for b in range(BPC):
                # ---- load x as bf16 (casting DMAs on gpsimd SWDGE),
                # chunked+interleaved so projection round i unblocks after
                # chunk i of both halves instead of after the full 2MB reads
                xbs = []
                for cc in range(2):
                    xb = xbfp.tile([128, N], BF16, tag="xb", name=f"xb{b}_{cc}")
                    xbs.append(xb)
                for i in range(NNCH):
                    sl = slice(i * NCH, (i + 1) * NCH)
                    for cc in range(2):
                        nc.gpsimd.dma_start(xbs[cc][:, sl],
                                            x_d[b, cc * 128:(cc + 1) * 128, sl])

                # ---- projections ----
                thph = projp.tile([128, N], BF16, tag="thph")  # 0:64 dup-theta, 64:128 dup-phi
                g_sb = projp.tile([C2, N], BF16, tag="g_sb")
                for i in range(NNCH):
                    sl = slice(i * NCH, (i + 1) * NCH)
                    ps1 = psb.tile([128, NCH], F32, tag="big", name=f"ps1_{b}_{i}")
                    for cc in range(2):
                        nc.tensor.matmul(ps1[:], wq[cc][:], xbs[cc][:, sl],
                                         start=(cc == 0), stop=(cc == 1))
                    nc.scalar.copy(thph[:, sl], ps1[:])
                    ps2 = psb.tile([128, NCH], F32, tag="big", name=f"ps2_{b}_{i}")
                    for cc in range(2):
                        nc.tensor.matmul(ps2[:], wg[cc][:], xbs[cc][:, sl],
                                         start=(cc == 0), stop=(cc == 1))
                    nc.scalar.copy(g_sb[:, sl], ps2[:])
                    pop_unit()
                    # fused w-pair maxpool for this 512-column chunk
                    if i == 0:
                        ph2t = projp.tile([64, N // 2], BF16, tag="ph2t")
                        g_t = projp.tile([C2, N // 2], BF16, tag="g_t")
                    csl = slice(i * 256, (i + 1) * 256)
                    pv = thph[:].rearrange("p (w2 two) -> p w2 two", two=2)
                    nc.vector.tensor_max(ph2t[:, csl], pv[64:128, csl, 0], pv[64:128, csl, 1])
                    pv2 = g_sb[:].rearrange("p (w2 two) -> p w2 two", two=2)
                    nc.vector.tensor_max(g_t[:, csl], pv2[:, csl, 0], pv2[:, csl, 1])
                th2 = thph[0:64]

                # ---- h-pair maxpool, chunked per mc so consumers start early ----
                ph2 = projp.tile([64, M], BF16, tag="ph2")
                gp = projp.tile([C2, M], BF16, tag="g_p")
                vph = ph2t[:].rearrange("p (h2 hb w2) -> p h2 w2 hb", h2=H // 2, hb=2, w2=W // 2)
                vg = g_t[:].rearrange("p (h2 hb w2) -> p h2 w2 hb", h2=H // 2, hb=2, w2=W // 2)
                for mc in range(MC):
                    h4 = slice(4 * mc, 4 * (mc + 1))
                    msl = slice(mc * 128, (mc + 1) * 128)
                    nc.vector.tensor_max(ph2[:, msl], vph[:, h4, :, 0], vph[:, h4, :, 1])
                    nc.vector.tensor_max(gp[:, msl], vg[:, h4, :, 0], vg[:, h4, :, 1])

                # gT transposes are emitted inside quarter 0 as PE fillers
                gts = []

                def emit_transpose(mc):
                    tp = psa.tile([128, 128], BF16, tag="a", name=f"tp{b}_{mc}")
                    nc.tensor.transpose(tp[:], gp[:, mc * 128:(mc + 1) * 128], id_b[:])
                    gt = gtp.tile([128, 128], BF16, tag=f"gt{mc}", name=f"gt{mc}_{b}")
                    nc.vector.tensor_copy(gt[:], tp[:])
                    gts.append(gt)

                # ---- scores/exp interleaved with attention at round granularity ----
                # PE executes its stream in order, so alternate 4 score MMs
                # (one round) with 8 attention-accumulation MMs; ScalarE exps
                # overlap the attention matmuls.
                ets = []
                for mc in range(MC):
                    et = expp.tile([128, N], BF16, tag=f"expT{mc}", name=f"expT{mc}_{b}")
                    ets.append(et)

                # deferred attention work-units, 8 accumulation MMs each:
                # (chunk, 'attn') and (chunk, 'den') + epilogue after 'den'
                aps_map = {}

                def unit_attn(i, b=b, ets=ets, gts=gts, aps_map=aps_map):
                    nsl = slice(i * NCH, (i + 1) * NCH)
                    aps = psa.tile([128, NCH], F32, tag="a", name=f"aps{b}_{i}")
                    aps_map[i] = aps
                    for mc in range(MC):
                        nc.tensor.matmul(aps[:], gts[mc][:], ets[mc][:, nsl],
                                         start=(mc == 0), stop=(mc == MC - 1),
                                         skip_group_check=True)

                def unit_den_epi(i, b=b, ets=ets, gts=gts, aps_map=aps_map):
                    nsl = slice(i * NCH, (i + 1) * NCH)
                    aps = aps_map.pop(i)
                    dps = psd.tile([128, NCH], F32, tag="d", name=f"dps{b}_{i}")
                    xr = xrp.tile([128, 1024], F32, tag="xr", name=f"xr{b}_{i}")
                    for oc in range(2):
                        nc.sync.dma_start(xr[:, oc * NCH:(oc + 1) * NCH],
                                          x_d[b, oc * 128:(oc + 1) * 128, nsl])
                    for mc in range(MC):
                        nc.tensor.matmul(dps[:], ones[:], ets[mc][:, nsl],
                                         start=(mc == 0), stop=(mc == MC - 1),
                                         skip_group_check=True)
                    rec = smallp.tile([128, NCH], F32, tag="rec", name=f"rec{b}_{i}")
                    nc.vector.reciprocal_approx_fast(rec[:], dps[:])
                    at = smallp.tile([128, NCH], BF16, tag="attn", name=f"at{b}_{i}")
                    nc.vector.scalar_tensor_tensor(
                        at[:], aps[:], 1.0, rec[:],
                        mybir.AluOpType.bypass, mybir.AluOpType.mult)
                    op0 = psa.tile([128, NCH], F32, tag="a", name=f"op0_{b}_{i}")
                    nc.tensor.matmul(op0[:], wo[:, 0:128], at[:], start=True, stop=True)
                    op1 = psd.tile([128, NCH], F32, tag="d", name=f"op1_{b}_{i}")
                    nc.tensor.matmul(op1[:], wo[:, 128:256], at[:], start=True, stop=True)
                    for oc, ops in ((0, op0), (1, op1)):
                        osb = outp.tile([128, NCH], F32, tag="osb",
                                        name=f"osb{b}_{i}_{oc}")
                        nc.vector.scalar_tensor_tensor(
                            osb[:], ops[:], 1.0, xr[:, oc * NCH:(oc + 1) * NCH],
                            mybir.AluOpType.bypass, mybir.AluOpType.add)
                        nc.sync.dma_start(out_d[b, oc * 128:(oc + 1) * 128, nsl],
                                          osb[:])

                for i in range(NNCH):
                    pending.append(lambda f=unit_attn, i=i: f(i))
                    pending.append(lambda f=unit_den_epi, i=i: f(i))

                for qt in range(5):
                    if qt < 4:
                        qsl = slice(qt * 1024, (qt + 1) * 1024)
                        for r in range(4):
                            mc_a, mc_b = 2 * r, 2 * r + 1
                            spa = psb.tile([128, 1024], F32, tag="big",
                                           name=f"spa{b}_{qt}_{r}")
                            spb = psb.tile([128, 1024], F32, tag="big",
                                           name=f"spb{b}_{qt}_{r}")
                            for hf in range(2):
                                nsl = slice(qt * 1024 + hf * 512, qt * 1024 + (hf + 1) * 512)
                                osl = slice(hf * 512, (hf + 1) * 512)
                                nc.tensor.matmul(
                                    spa[:, osl], ph2[0:32, mc_a * 128:(mc_a + 1) * 128],
                                    th2[0:32, nsl], start=True, stop=True)
                                nc.tensor.matmul(
                                    spb[:, osl], ph2[32:64, mc_b * 128:(mc_b + 1) * 128],
                                    th2[32:64, nsl], start=True, stop=True)
                            nc.scalar.activation(ets[mc_a][:, qsl], spa[:],
                                                 mybir.ActivationFunctionType.Exp)
                            nc.scalar.activation(ets[mc_b][:, qsl], spb[:],
                                                 mybir.ActivationFunctionType.Exp)
                            if qt == 0:
                                emit_transpose(2 * r)
                                emit_transpose(2 * r + 1)
                            else:
                                pop_unit()
                    else:
                        keep = 4 if b == 0 else 0
                        while len(pending) > keep:
                            pop_unit()

    nc.compile()
    return nc


_NC_CACHE = None


def _get_nc():
    global _NC_CACHE
    if _NC_CACHE is None:
        _NC_CACHE = build_kernel()
    return _NC_CACHE


def prep_inputs(x, w_theta, w_phi, w_g, w_o, gamma):
    """Host-side prep: shard x over 8 cores; transpose/scale/pack weights."""
    x = np.asarray(x, dtype=np.float32).reshape(B, C, N)
    w_theta = np.asarray(w_theta, dtype=np.float32)
    w_phi = np.asarray(w_phi, dtype=np.float32)
    w_g = np.asarray(w_g, dtype=np.float32)
    w_o = np.asarray(w_o, dtype=np.float32)
    gamma = np.float32(gamma)

    # combined projection weight: [th th ph ph] along output dim
    wqT = np.concatenate([w_theta.T, w_theta.T, w_phi.T, w_phi.T], axis=1)  # [256,128]
    wq = np.ascontiguousarray(wqT.reshape(2, 128, 128))
    wgq = np.ascontiguousarray(w_g.T.reshape(2, 128, C2))
    wo = np.ascontiguousarray((gamma * w_o).T)
    ident = np.eye(128, dtype=np.float32)

    in_maps = []
    for core in range(NCORES):
        shard = np.ascontiguousarray(x[core * BPC:(core + 1) * BPC])
        in_maps.append({"x": shard, "wq": wq, "wg": wgq, "wo": wo, "ident": ident})
    return in_maps


def run(inputs, trace=False, **kw):
    nc = _get_nc()
    in_maps = prep_inputs(**inputs)
    res = run_bass_kernel_spmd(nc, in_maps, core_ids=list(range(NCORES)),
                               trace=trace, **kw)
    outs = [res.results[i]["out"] for i in range(NCORES)]
    full = np.concatenate(outs, axis=0).reshape(B, C, H, W).astype(np.float32)
    return full, res


def kernel(**inputs):
    full, _ = run(inputs, trace=False)
    return full
